# revision 1
# baseline (speedup 1.0000x reference)
"""Trainium2 Bass kernel for nn_AttnBlock (GroupNorm -> 1x1 q/k/v -> attention -> proj -> residual).

Input x: [4, 512, 64, 64] f32. Sharding: 8 cores = 4 batches x 2 query-halves.
Each core gets its batch's full x (columns permuted so its query half is first),
computes GroupNorm + full k/vT, q for its half, attention over all 4096 keys for
its 2048 queries, proj + residual, and returns [512, 2048].

Numerics: GroupNorm stats and softmax normalization in f32; all matmuls in
fp8e4m3 with DoubleRow packing (2x PE throughput), accumulating in f32 PSUM.
exp(s - 1) keeps attention weights inside the e4m3 normal range; attn@v stays
un-normalized (scaled 1/512 into fp8) and the 512/denominator factor is applied
after the output projection (division commutes with the channel mixing).

Layouts (per core):
  x_sb  [128, 4, 4096]    bf16  (stats + hn source; f32 x streamed for residual)
  k_f8  [128, 2, 2, 4096] fp8   c-pair-packed lhsT for scores^T
  q_f8  [128, 2, 2, 2048] fp8   c-pair-packed rhs for scores^T
  vT_f8 [128, 16, 2, 512] fp8   j-pair-packed lhsT for attn@v
Attention runs in scores^T[j, i] layout; the softmax denominator is a
DoubleRow ones-matmul (partition reduction on PE). fp8 attn tiles persist per
query chunk so attn@v runs ct-major in 2 PSUM banks, freeing banks to give
phase-1 and attention disjoint PSUM tags (phases overlap in the schedule).
"""

import numpy as np
import ml_dtypes

import concourse.bass as bass
import concourse.mybir as mybir
import concourse.tile as tile
from concourse.vector_clock import ScopedClock
from concourse.bass_utils import run_bass_kernel_spmd

F32 = mybir.dt.float32
F32R = mybir.dt.float32r
BF16 = mybir.dt.bfloat16
FP8 = mybir.dt.float8e4
AF = mybir.ActivationFunctionType
ALU = mybir.AluOpType

P = 128
C = 512          # channels
N = 4096         # spatial positions (64*64)
NQ = 2048        # queries per core (half)
CT = C // P      # 4 channel tiles
JC = N // 512    # 8 key chunks of 512
JT = N // P      # 32 key tiles of 128
ICH = NQ // 512  # 4 query chunks of 512
NUM_GROUPS = 16
GSIZE = C // NUM_GROUPS            # 32 channels per group
G_ELEMS = GSIZE * N                # elements per group
EPS = 1e-6
SCALE = float(C) ** -0.5


class PatchedTileContext(tile.TileContext):
    """walrus in this container accepts only ONE sync-wait per instruction;
    split extra waits onto same-engine NoOps placed just before the
    instruction (same queue => waits still execute before it)."""

    def _lower_ordered_insts(self, ordered):
        for bb_name, insts in list(ordered.items()):
            new_list = []
            for inst in insts:
                si = inst.sync_info
                if si is not None and si.on_wait and len(si.on_wait) > 1:
                    waits = list(si.on_wait)
                    for w in waits[:-1]:
                        nop = mybir.InstNoOp(
                            name=self.nc.get_next_instruction_name(),
                            engine=inst.engine,
                            sync_info=mybir.SyncInfo(on_wait=[w], on_update=[]),
                            bass_nofuse=True,
                        )
                        new_list.append(nop)
                    si.on_wait = [waits[-1]]
                new_list.append(inst)
            ordered[bb_name] = new_list
        super()._lower_ordered_insts(ordered)

    def _drain_and_barrier(self, tick_clock, wait_clock):
        drain_inst = self.nc.sync.drain()
        wait_clock.add_sem_waits(
            drain_inst.ins, ScopedClock({None: tick_clock.global_clock})
        )
        si = drain_inst.ins.sync_info
        if si is not None and si.on_wait and len(si.on_wait) > 1:
            waits = list(si.on_wait)
            si.on_wait = [waits[0]]
            for w in waits[1:]:
                d2 = self.nc.sync.drain()
                d2.ins.sync_info = mybir.SyncInfo(on_wait=[w], on_update=[])
        self.nc.all_engine_barrier()
        assert self.sems is not None
        popped = self.nc._tile_sem_poison_stack.pop()
        assert popped is self._sem_poison
        self.nc.clear_and_free_semaphores(list(self.sems.allocated().values()))
        self.nc.all_engine_barrier()


def build_nc(reps=1):
    nc = bass.Bass(name=f"attnblk_r{reps}")

    x_d = nc.dram_tensor("x", [C, N], F32, kind="ExternalInput")
    xbf_d = nc.dram_tensor("xbf", [C, N], BF16, kind="ExternalInput")
    wqtf8_d = nc.dram_tensor("wqtf8", [P, 4 * 512], FP8, kind="ExternalInput")
    wktf8_d = nc.dram_tensor("wktf8", [P, 4 * 512], FP8, kind="ExternalInput")
    wvtf8_d = nc.dram_tensor("wvtf8", [P, 4 * 512], FP8, kind="ExternalInput")
    wptf8_d = nc.dram_tensor("wptf8", [P, 4 * 512], FP8, kind="ExternalInput")
    gamma_d = nc.dram_tensor("gamma", [C], F32, kind="ExternalInput")
    beta_d = nc.dram_tensor("beta", [C], F32, kind="ExternalInput")
    bq_d = nc.dram_tensor("bq", [C], F32, kind="ExternalInput")
    bk_d = nc.dram_tensor("bk", [C], F32, kind="ExternalInput")
    bv_d = nc.dram_tensor("bv", [C], F32, kind="ExternalInput")
    bp_d = nc.dram_tensor("bp", [C], F32, kind="ExternalInput")
    g4_d = nc.dram_tensor("g4", [P, 4], F32, kind="ExternalInput")
    g4t_d = nc.dram_tensor("g4t", [4, P], F32, kind="ExternalInput")
    onesr_d = nc.dram_tensor("onesr", [1, P], F32R, kind="ExternalInput")
    out_d = nc.dram_tensor("out", [C, NQ], F32, kind="ExternalOutput")

    with PatchedTileContext(nc) as tc:
        with (
            tc.tile_pool(name="const", bufs=1) as const,
            tc.tile_pool(name="persist", bufs=1) as persist,
            tc.tile_pool(name="small", bufs=4) as small,
            tc.tile_pool(name="hnp", bufs=3) as hnp,
            tc.tile_pool(name="atp", bufs=34) as atp,
            tc.tile_pool(name="o2np", bufs=2) as o2np,
            tc.tile_pool(name="finp", bufs=3) as finp,
            tc.tile_pool(name="ps", bufs=1, space="PSUM") as ps,
        ):
            # ---------------- constants ----------------
            wqt_f8 = const.tile([P, 2, 2, C], FP8)
            nc.gpsimd.dma_start(wqt_f8[:], wqtf8_d[:, :].rearrange("p (kp s co) -> p kp s co", kp=2, s=2))
            wkt_f8 = const.tile([P, 2, 2, C], FP8)
            nc.gpsimd.dma_start(wkt_f8[:], wktf8_d[:, :].rearrange("p (kp s co) -> p kp s co", kp=2, s=2))
            wvt_f8 = const.tile([P, 2, 2, C], FP8)
            nc.gpsimd.dma_start(wvt_f8[:], wvtf8_d[:, :].rearrange("p (kp s co) -> p kp s co", kp=2, s=2))
            wpt_f8 = const.tile([P, 2, 2, C], FP8)
            nc.gpsimd.dma_start(wpt_f8[:], wptf8_d[:, :].rearrange("p (kp s co) -> p kp s co", kp=2, s=2))

            gam = const.tile([P, CT], F32)
            nc.gpsimd.dma_start(gam[:], gamma_d[:].rearrange("(t p) -> p t", p=P))
            bet = const.tile([P, CT], F32)
            nc.gpsimd.dma_start(bet[:], beta_d[:].rearrange("(t p) -> p t", p=P))
            bq4 = const.tile([P, CT], F32)
            nc.gpsimd.dma_start(bq4[:], bq_d[:].rearrange("(t p) -> p t", p=P))
            bk4 = const.tile([P, CT], F32)
            nc.gpsimd.dma_start(bk4[:], bk_d[:].rearrange("(t p) -> p t", p=P))
            bp4 = const.tile([P, CT], F32)
            nc.gpsimd.dma_start(bp4[:], bp_d[:].rearrange("(t p) -> p t", p=P))
            g4_sb = const.tile([P, 4], F32)
            nc.gpsimd.dma_start(g4_sb[:], g4_d[:, :])
            g4t_sb = const.tile([4, P], F32)
            nc.gpsimd.dma_start(g4t_sb[:], g4t_d[:, :])

            ones_row = const.tile([1, P], F32R)
            nc.gpsimd.dma_start(ones_row[:], onesr_d[:, :])
            eps_sb = const.tile([P, 1], F32)
            nc.vector.memset(eps_sb[:], EPS)
            bias_m1 = const.tile([P, 1], F32)
            nc.vector.memset(bias_m1[:], -1.0)
            ones_f8 = const.tile([P, 2, 16], FP8)
            nc.vector.memset(ones_f8[:], 1.0)

            # bv broadcast [128, 512] (v bias lives on the free dim of vT)
            bvb = persist.tile([P, C], F32)
            nc.sync.dma_start(
                bvb[:], bv_d[:].rearrange("(a c) -> a c", a=1).to_broadcast([P, C])
            )

            # ---------------- x resident (bf16: stats + hn inputs) ----------------
            x_sb = persist.tile([P, CT, N], BF16)

            k_f8 = persist.tile([P, CT // 2, 2, N], FP8)
            vT_f8 = persist.tile([P, JT // 2, 2, 512], FP8)
            q_f8 = persist.tile([P, CT // 2, 2, NQ], FP8)
            scale_sb = persist.tile([P, CT], F32)
            bias_sb = persist.tile([P, CT], F32)

            for _rep in range(reps):
              for ct in range(CT):
                  for xh in range(2):
                      eng = nc.sync if (2 * ct + xh) % 2 == 0 else nc.scalar
                      eng.dma_start(
                          x_sb[:, ct, xh * 2048:(xh + 1) * 2048],
                          xbf_d[ct * P:(ct + 1) * P, xh * 2048:(xh + 1) * 2048],
                      )
              if True:
                  # ---------------- phase 0: groupnorm stats ----------------
                  # red per ct = (mean_c, E_c[x^2]) [P, 2]; g4 is host-scaled by
                  # 1/GSIZE so the group matmul directly yields (mu_g, E_g[x^2]).
                  # ct0 computed on ACT (sum + sumsq accum), ct1-3 on DVE (bn_stats).
                  mrall = small.tile([4, 8], F32, tag="mrall")
                  for ct in range(CT):
                      red = small.tile([P, 2], F32, tag="red", name=f"red_{ct}")
                      if ct == 0:
                          # ACT route: sums/sumsq accumulate while the x DMA streams
                          reds = small.tile([P, JC], F32, tag="reds")
                          redq = small.tile([P, JC], F32, tag="redq")
                          for jc in range(JC):
                              cp = hnp.tile([P, 512], F32, tag="cp", name=f"cp_{jc}")
                              nc.scalar.activation(
                                  cp[:], x_sb[:, ct, jc * 512:(jc + 1) * 512], AF.Copy,
                                  accum_out=reds[:, jc:jc + 1],
                              )
                              sq = hnp.tile([P, 512], F32, tag="sq", name=f"sq_{jc}")
                              nc.scalar.activation(
                                  sq[:], x_sb[:, ct, jc * 512:(jc + 1) * 512], AF.Square,
                                  accum_out=redq[:, jc:jc + 1],
                              )
                          rsum = small.tile([P, 2], F32, tag="rsum")
                          nc.vector.reduce_sum(rsum[:, 0:1], reds[:], axis=mybir.AxisListType.X)
                          nc.vector.reduce_sum(rsum[:, 1:2], redq[:], axis=mybir.AxisListType.X)
                          nc.vector.tensor_scalar_mul(red[:], rsum[:], 1.0 / N)
                      else:
                          bnst = small.tile([P, JC, 6], F32, tag="bnst", name=f"bnst_{ct}")
                          for jc in range(JC):
                              nc.vector.bn_stats(bnst[:, jc, :], x_sb[:, ct, jc * 512:(jc + 1) * 512])
                          mv = small.tile([P, 2], F32, tag="mv", name=f"mv_{ct}")
                          nc.vector.bn_aggr(mv[:], bnst[:])
                          msq = small.tile([P, 1], F32, tag="msq", name=f"msq_{ct}")
                          nc.scalar.activation(msq[:], mv[:, 0:1], AF.Square)
                          nc.scalar.copy(red[:, 0:1], mv[:, 0:1])
                          nc.vector.tensor_tensor(red[:, 1:2], mv[:, 1:2], msq[:], ALU.add)
                      gps = ps.tile([4, 2], F32, tag="den", bufs=1, name=f"gps_{ct}")
                      nc.tensor.matmul(
                          gps[:], lhsT=g4_sb[:], rhs=red[:],
                          start=True, stop=True,
                      )
                      nc.scalar.copy(mrall[:, ct:ct + 1], gps[:, 0:1])
                      nc.scalar.copy(mrall[:, 4 + ct:5 + ct], gps[:, 1:2])
                  # mu = mrall[:, :4]; var = mrall[:, 4:] - mu^2 (batched)
                  musq = small.tile([4, 4], F32, tag="musq")
                  nc.scalar.activation(musq[:], mrall[:, 0:4], AF.Square)
                  var4 = small.tile([4, 4], F32, tag="var4")
                  nc.vector.tensor_tensor(var4[:], mrall[:, 4:8], musq[:], ALU.subtract)
                  std4 = small.tile([4, 4], F32, tag="std4")
                  nc.scalar.activation(std4[:], var4[:], AF.Sqrt, bias=eps_sb[0:4, :])
                  nc.vector.reciprocal(mrall[:, 4:8], std4[:])
                  # one bcast matmul: [128, 8] = (mu | rstd) per channel
                  mrp = ps.tile([P, 8], F32, tag="den", bufs=1, name="mrp")
                  nc.tensor.matmul(
                      mrp[:], lhsT=g4t_sb[:], rhs=mrall[:],
                      start=True, stop=True,
                  )
                  # scale = gamma * rstd ; bias = beta - mu * scale (batched)
                  nc.vector.tensor_tensor(scale_sb[:], gam[:], mrp[:, 4:8], ALU.mult)
                  tb = small.tile([P, 4], F32, tag="tb")
                  nc.vector.tensor_tensor(tb[:], mrp[:, 0:4], scale_sb[:], ALU.mult)
                  nc.vector.tensor_tensor(bias_sb[:], bet[:], tb[:], ALU.subtract)

                  # ---------------- phase 1: hn -> k, vT, q ----------------
                  for jc in range(JC):
                      hn8 = hnp.tile([P, 2, 2, 512], FP8, tag="hn8")
                      for kc in range(CT):
                          nc.vector.tensor_scalar(
                              hn8[:, kc // 2, kc % 2, :], x_sb[:, kc, jc * 512:(jc + 1) * 512],
                              scale_sb[:, kc:kc + 1], bias_sb[:, kc:kc + 1],
                              ALU.mult, ALU.add,
                          )
                      for co in range(CT):
                          pk = ps.tile([P, 512], F32, tag="pp", bufs=2, name="pk")
                          for kp in range(2):
                              nc.tensor.matmul(
                                  pk[:], lhsT=wkt_f8[:, kp, :, co * P:(co + 1) * P], rhs=hn8[:, kp],
                                  perf_mode=mybir.MatmulPerfMode.DoubleRow,
                                  start=(kp == 0), stop=(kp == 1),
                              )
                          nc.scalar.activation(
                              k_f8[:, co // 2, co % 2, jc * 512:(jc + 1) * 512], pk[:],
                              AF.Identity, bias=bk4[:, co:co + 1],
                          )
                      for jl in range(4):
                          jt = jc * 4 + jl
                          pv = ps.tile([P, 512], F32, tag="pp", bufs=2, name="pv")
                          for kp in range(2):
                              nc.tensor.matmul(
                                  pv[:], lhsT=hn8[:, kp, :, jl * P:(jl + 1) * P], rhs=wvt_f8[:, kp],
                                  perf_mode=mybir.MatmulPerfMode.DoubleRow,
                                  start=(kp == 0), stop=(kp == 1),
                              )
                          nc.vector.tensor_tensor(vT_f8[:, jt // 2, jt % 2, :], pv[:], bvb[:], ALU.add)
                      if jc < ICH:
                          for co in range(CT):
                              pq = ps.tile([P, 512], F32, tag="pp", bufs=2, name="pq")
                              for kp in range(2):
                                  nc.tensor.matmul(
                                      pq[:], lhsT=wqt_f8[:, kp, :, co * P:(co + 1) * P], rhs=hn8[:, kp],
                                      perf_mode=mybir.MatmulPerfMode.DoubleRow,
                                      start=(kp == 0), stop=(kp == 1),
                                  )
                              nc.vector.tensor_scalar(
                                  q_f8[:, co // 2, co % 2, jc * 512:(jc + 1) * 512], pq[:],
                                  bq4[:, co:co + 1], None, ALU.add,
                              )

              # ---------------- phase 2: attention ----------------
              if True:
                  for ich in range(ICH):
                      den = ps.tile([1, 512], F32, tag="den", bufs=1, name=f"den_{ich}")
                      # residual + out-proj bias staged early, off the critical path
                      xqb = o2np.tile([P, CT, 512], F32, tag="xqb")
                      for ot in range(CT):
                          xq = finp.tile([P, 512], F32, tag="xq", name=f"xq_{ich}_{ot}")
                          nc.sync.dma_start(
                              xq[:], x_d[ot * P:(ot + 1) * P, ich * 512:(ich + 1) * 512]
                          )
                          nc.gpsimd.tensor_scalar(
                              xqb[:, ot, :], xq[:], bp4[:, ot:ot + 1], None, ALU.add
                          )
                      at2s = []
                      for t in range(JT // 2):
                          at2 = atp.tile([P, 2, 512], FP8, tag="at", name=f"at2_{ich}_{t}")
                          at2s.append(at2)
                          for s in range(2):
                              jt = 2 * t + s
                              pssc = ps.tile([P, 512], F32, tag="sc", bufs=3, name="pssc")
                              for kp in range(CT // 2):
                                  nc.tensor.matmul(
                                      pssc[:], lhsT=k_f8[:, kp, :, jt * P:(jt + 1) * P],
                                      rhs=q_f8[:, kp, :, ich * 512:(ich + 1) * 512],
                                      perf_mode=mybir.MatmulPerfMode.DoubleRow,
                                      start=(kp == 0), stop=(kp == CT // 2 - 1),
                                  )
                              nc.scalar.activation(at2[:, s, :], pssc[:], AF.Exp, scale=SCALE, bias=bias_m1[:])
                          nc.tensor.matmul(
                              den[:], lhsT=ones_f8[:, :, 0:1], rhs=at2[:],
                              perf_mode=mybir.MatmulPerfMode.DoubleRow,
                              start=(t == 0), stop=(t == JT // 2 - 1),
                          )
                      # attn@v over the persisted fp8 tiles, un-normalized
                      # (1/denominator applied after proj). ct-major uses one o2
                      # bank at a time; the final chunk goes tile-major across 4
                      # banks (2 borrowed from the retired phase-1 tag) so its
                      # tail is not serialized behind the full j-loop.
                      o2n = o2np.tile([P, 2, 2, 512], FP8, tag="o2n")
                      if ich < ICH - 1:
                          for ct in range(CT):
                              o2t = ps.tile([P, 512], F32, tag="o2", bufs=2, name=f"o2_{ich}_{ct}")
                              for t in range(JT // 2):
                                  nc.tensor.matmul(
                                      o2t[:], lhsT=vT_f8[:, t, :, ct * P:(ct + 1) * P], rhs=at2s[t][:],
                                      perf_mode=mybir.MatmulPerfMode.DoubleRow,
                                      start=(t == 0), stop=(t == JT // 2 - 1),
                                  )
                              nc.vector.tensor_scalar(
                                  o2n[:, ct // 2, ct % 2, :], o2t[:], 1.0 / 512.0, None, ALU.mult
                              )
                      else:
                          o2l = [
                              ps.tile([P, 512], F32, tag=("o2" if i < 2 else "pp"),
                                      bufs=2, name=f"o2l_{i}")
                              for i in range(CT)
                          ]
                          for t in range(JT // 2):
                              for ct in range(CT):
                                  nc.tensor.matmul(
                                      o2l[ct][:], lhsT=vT_f8[:, t, :, ct * P:(ct + 1) * P],
                                      rhs=at2s[t][:],
                                      perf_mode=mybir.MatmulPerfMode.DoubleRow,
                                      start=(t == 0), stop=(t == JT // 2 - 1),
                                  )
                          for ct in range(CT):
                              if ct % 2 == 0:
                                  nc.scalar.mul(o2n[:, ct // 2, ct % 2, :], o2l[ct][:], 1.0 / 512.0)
                              else:
                                  nc.vector.tensor_scalar(
                                      o2n[:, ct // 2, ct % 2, :], o2l[ct][:], 1.0 / 512.0, None, ALU.mult
                                  )
                      rec = small.tile([1, 512], F32R, tag="rec")
                      with nc.allow_low_precision(reason="f32r softmax denom reciprocal"):
                          nc.vector.reciprocal(rec[:], den[:])
                      rbp = ps.tile([P, 512], F32, tag="pp", bufs=2, name=f"rbp_{ich}")
                      nc.tensor.matmul(rbp[:], lhsT=ones_row[:], rhs=rec[:], start=True, stop=True)
                      rb = finp.tile([P, 512], F32, tag="rb")
                      nc.vector.tensor_copy(rb[:], rbp[:])
                      for ot in range(CT):
                          p3 = ps.tile([P, 512], F32, tag="pp", bufs=2, name="p3")
                          for kp in range(2):
                              nc.tensor.matmul(
                                  p3[:], lhsT=wpt_f8[:, kp, :, ot * P:(ot + 1) * P], rhs=o2n[:, kp],
                                  perf_mode=mybir.MatmulPerfMode.DoubleRow,
                                  start=(kp == 0), stop=(kp == 1),
                              )
                          fin = finp.tile([P, 512], F32, tag="fin")
                          nc.vector.tensor_tensor(fin[:], p3[:], rb[:], ALU.mult)
                          nc.vector.tensor_tensor(fin[:], fin[:], xqb[:, ot, :], ALU.add)
                          nc.sync.dma_start(
                              out_d[ot * P:(ot + 1) * P, ich * 512:(ich + 1) * 512], fin[:]
                          )
    return nc


_NC = None


def _get_nc():
    global _NC
    if _NC is None:
        _NC = build_nc()
    return _NC


def _make_in_maps(x, gamma, beta, wq, bq, wk, bk, wv, bv, wp, bp):
    x = np.ascontiguousarray(np.asarray(x, dtype=np.float32)).reshape(4, C, N)
    bf = ml_dtypes.bfloat16
    def pack8(w):
        return np.ascontiguousarray(
            np.asarray(w, np.float32).T.reshape(2, 2, P, 512).transpose(2, 0, 1, 3)
            .reshape(P, 4 * 512).astype(mybir.dt.np(FP8))
        )


    g4i = np.zeros((P, 4), np.float32)
    for p in range(P):
        g4i[p, p // GSIZE] = 1.0
    g4 = g4i / GSIZE          # group-mean matmul (pre-scaled)
    g4t = np.ascontiguousarray(g4i.T)  # broadcast indicator (0/1)
    common = {
        "wqtf8": pack8(wq), "wktf8": pack8(wk),
        "wptf8": np.ascontiguousarray(
            np.asarray(wp, np.float32).T.reshape(2, 2, P, 512).transpose(2, 0, 1, 3)
            .reshape(P, 4 * 512).astype(mybir.dt.np(FP8))
        ),
        "wvtf8": np.ascontiguousarray(
            np.asarray(wv, np.float32).T.reshape(2, 2, P, 512).transpose(2, 0, 1, 3)
            .reshape(P, 4 * 512).astype(mybir.dt.np(FP8))
        ),
        "gamma": np.asarray(gamma, np.float32), "beta": np.asarray(beta, np.float32),
        "bq": np.asarray(bq, np.float32), "bk": np.asarray(bk, np.float32),
        "bv": np.asarray(bv, np.float32), "bp": np.asarray(bp, np.float32),
        "g4": g4, "g4t": g4t,
        "onesr": np.full((1, P), 512.0, np.float32),
    }
    in_maps = []
    for core in range(8):
        bidx, half = core // 2, core % 2
        xb = x[bidx]
        if half == 0:
            xp = xb
        else:
            xp = np.concatenate([xb[:, NQ:], xb[:, :NQ]], axis=1)
        xp = np.ascontiguousarray(xp)
        in_maps.append({"x": xp, "xbf": xp.astype(bf), **common})
    return in_maps


def run(inputs, trace=False):
    nc = _get_nc()
    in_maps = _make_in_maps(**inputs)
    res = run_bass_kernel_spmd(nc, in_maps, list(range(8)), trace=trace)
    out = np.empty((4, C, N), np.float32)
    for core in range(8):
        bidx, half = core // 2, core % 2
        o = res.results[core]["out"]
        if half == 0:
            out[bidx, :, :NQ] = o
        else:
            out[bidx, :, NQ:] = o
    return out.reshape(4, C, 64, 64), res


def kernel(**inputs):
    out, _ = run(inputs, trace=False)
    return out



# revision 78
# speedup vs baseline: 1.2939x; 1.2939x over previous
"""Trainium2 Bass kernel for nn_AttnBlock (GroupNorm -> 1x1 q/k/v -> attention -> proj -> residual).

Input x: [4, 512, 64, 64] f32. Sharding: 8 cores = 4 batches x 2 query-halves.
Each core gets its batch's full x (columns permuted so its query half is first),
computes GroupNorm + full k/vT, q for its half, attention over all 4096 keys for
its 2048 queries, proj + residual, and returns [512, 2048].

Numerics: GroupNorm stats and softmax normalization in f32; all matmuls in
fp8e4m3 with DoubleRow packing (2x PE throughput), accumulating in f32 PSUM.
exp(s - 1) keeps attention weights inside the e4m3 normal range.

Bias algebra (exact):
  - bk is dropped entirely: softmax over keys is invariant to the
    per-query-constant term (q+bq)@bk.
  - bv is folded into the output-proj bias on the host: rows of the
    normalized attention sum to 1, so attn@(v+bv) = attn@v_raw + bv and
    out = wp@(attn@v_raw)/den + (bp + wp@bv).
  - 1/den is applied at the attn@v PSUM->fp8 conversion (o2n = o2t * rb),
    so the final step is a single add of the residual+bias tile.

Schedule (engines run their streams in order, so overlap is by emission):
  GN stats -> phase A (hn + k/q: Pool k converts, DVE q converts)
  -> phase B (v pairs interleaved with ich0 scores/exp: Pool v converts)
  -> stages ich=1..3: scores/exp(ich) interleaved on PE with attnv(ich-1)
     chunks, proj(ich-1) at stage end  -> flush attnv/proj(ich3).
ACT runs only the 64 paired exps ([128,2,512] over a 2-bank PSUM pair) plus
GN ct0 accumulators. PSUM: score/qkv pairs 2x2 banks, attn@v 2, den 1,
rbp/proj 1.
"""

import numpy as np
import ml_dtypes

import concourse.bass as bass
import concourse.mybir as mybir
import concourse.tile as tile
from concourse.vector_clock import ScopedClock
from concourse.bass_utils import run_bass_kernel_spmd

F32 = mybir.dt.float32
F32R = mybir.dt.float32r
BF16 = mybir.dt.bfloat16
FP8 = mybir.dt.float8e4
AF = mybir.ActivationFunctionType
ALU = mybir.AluOpType

P = 128
C = 512          # channels
N = 4096         # spatial positions (64*64)
NQ = 2048        # queries per core (half)
CT = C // P      # 4 channel tiles
JC = N // 512    # 8 key chunks of 512
JT = N // P      # 32 key tiles of 128
ICH = NQ // 512  # 4 query chunks of 512
NUM_GROUPS = 16
GSIZE = C // NUM_GROUPS            # 32 channels per group
EPS = 1e-6
SCALE = float(C) ** -0.5
DR = mybir.MatmulPerfMode.DoubleRow


class PatchedTileContext(tile.TileContext):
    """walrus in this container accepts only ONE sync-wait per instruction;
    split extra waits onto same-engine NoOps placed just before the
    instruction (same queue => waits still execute before it)."""

    def _lower_ordered_insts(self, ordered):
        for bb_name, insts in list(ordered.items()):
            new_list = []
            for inst in insts:
                si = inst.sync_info
                if si is not None and si.on_wait and len(si.on_wait) > 1:
                    waits = list(si.on_wait)
                    for w in waits[:-1]:
                        nop = mybir.InstNoOp(
                            name=self.nc.get_next_instruction_name(),
                            engine=inst.engine,
                            sync_info=mybir.SyncInfo(on_wait=[w], on_update=[]),
                            bass_nofuse=True,
                        )
                        new_list.append(nop)
                    si.on_wait = [waits[-1]]
                new_list.append(inst)
            ordered[bb_name] = new_list
        super()._lower_ordered_insts(ordered)

    def _drain_and_barrier(self, tick_clock, wait_clock):
        drain_inst = self.nc.sync.drain()
        wait_clock.add_sem_waits(
            drain_inst.ins, ScopedClock({None: tick_clock.global_clock})
        )
        si = drain_inst.ins.sync_info
        if si is not None and si.on_wait and len(si.on_wait) > 1:
            waits = list(si.on_wait)
            si.on_wait = [waits[0]]
            for w in waits[1:]:
                d2 = self.nc.sync.drain()
                d2.ins.sync_info = mybir.SyncInfo(on_wait=[w], on_update=[])
        self.nc.all_engine_barrier()
        assert self.sems is not None
        popped = self.nc._tile_sem_poison_stack.pop()
        assert popped is self._sem_poison
        self.nc.clear_and_free_semaphores(list(self.sems.allocated().values()))
        self.nc.all_engine_barrier()


def build_nc(reps=1):
    nc = bass.Bass(name=f"attnblk_r{reps}")

    x_d = nc.dram_tensor("x", [C, N], F32, kind="ExternalInput")
    xbf_d = nc.dram_tensor("xbf", [C, N], BF16, kind="ExternalInput")
    wqtf8_d = nc.dram_tensor("wqtf8", [P, 4 * 512], FP8, kind="ExternalInput")
    wvtf8_d = nc.dram_tensor("wvtf8", [P, 4 * 512], FP8, kind="ExternalInput")
    wptf8_d = nc.dram_tensor("wptf8", [P, 4 * 512], FP8, kind="ExternalInput")
    gamma_d = nc.dram_tensor("gamma", [C], F32, kind="ExternalInput")
    beta_d = nc.dram_tensor("beta", [C], F32, kind="ExternalInput")
    bq_d = nc.dram_tensor("bq", [C], F32, kind="ExternalInput")
    bp_d = nc.dram_tensor("bp", [C], F32, kind="ExternalInput")
    g4_d = nc.dram_tensor("g4", [P, 4], F32, kind="ExternalInput")
    g4t_d = nc.dram_tensor("g4t", [4, P], F32, kind="ExternalInput")
    onesr_d = nc.dram_tensor("onesr", [1, P], F32R, kind="ExternalInput")
    out_d = nc.dram_tensor("out", [C, NQ], F32, kind="ExternalOutput")

    with PatchedTileContext(nc) as tc:
        with (
            tc.tile_pool(name="const", bufs=1) as const,
            tc.tile_pool(name="persist", bufs=1) as persist,
            tc.tile_pool(name="small", bufs=4) as small,
            tc.tile_pool(name="hnp", bufs=3) as hnp,
            tc.tile_pool(name="atp", bufs=34) as atp,
            tc.tile_pool(name="o2np", bufs=2) as o2np,
            tc.tile_pool(name="finp", bufs=3) as finp,
            tc.tile_pool(name="ps", bufs=1, space="PSUM") as ps,
        ):
            # ---------------- persistent tiles ----------------
            x_sb = persist.tile([P, CT, N], BF16)

            # SP queue: x ct0 (ACT stats route) first in fine chunks, GN matmul
            # consts, x ct3.
            for xh in range(4):
                nc.sync.dma_start(
                    x_sb[:, 0, xh * 1024:(xh + 1) * 1024],
                    xbf_d[0:P, xh * 1024:(xh + 1) * 1024],
                )
            g4_sb = const.tile([P, 4], F32)
            nc.sync.dma_start(g4_sb[:], g4_d[:, :])
            g4t_sb = const.tile([4, P], F32)
            nc.sync.dma_start(g4t_sb[:], g4t_d[:, :])
            ones_row = const.tile([1, P], F32R)
            nc.sync.dma_start(ones_row[:], onesr_d[:, :])
            for xh in range(2):
                nc.sync.dma_start(
                    x_sb[:, 3, xh * 2048:(xh + 1) * 2048],
                    xbf_d[3 * P:4 * P, xh * 2048:(xh + 1) * 2048],
                )
            # Pool queue: x ct1/ct2 (DVE bn_stats starts with ct1), then consts.
            for ct in (1, 2):
                for xh in range(2):
                    nc.gpsimd.dma_start(
                        x_sb[:, ct, xh * 2048:(xh + 1) * 2048],
                        xbf_d[ct * P:(ct + 1) * P, xh * 2048:(xh + 1) * 2048],
                    )
            gam = const.tile([P, CT], F32)
            nc.gpsimd.dma_start(gam[:], gamma_d[:].rearrange("(t p) -> p t", p=P))
            bet = const.tile([P, CT], F32)
            nc.gpsimd.dma_start(bet[:], beta_d[:].rearrange("(t p) -> p t", p=P))
            bq4 = const.tile([P, CT], F32)
            nc.gpsimd.dma_start(bq4[:], bq_d[:].rearrange("(t p) -> p t", p=P))
            bp4 = const.tile([P, CT], F32)
            nc.gpsimd.dma_start(bp4[:], bp_d[:].rearrange("(t p) -> p t", p=P))
            wqt_f8 = const.tile([P, 2, 2, C], FP8)
            nc.gpsimd.dma_start(wqt_f8[:], wqtf8_d[:, :].rearrange("p (kp s co) -> p kp s co", kp=2, s=2))
            wvt_f8 = const.tile([P, 2, 2, C], FP8)
            nc.gpsimd.dma_start(wvt_f8[:], wvtf8_d[:, :].rearrange("p (kp s co) -> p kp s co", kp=2, s=2))
            wpt_f8 = const.tile([P, 2, 2, C], FP8)
            nc.gpsimd.dma_start(wpt_f8[:], wptf8_d[:, :].rearrange("p (kp s co) -> p kp s co", kp=2, s=2))

            bias_m1 = const.tile([P, 1], F32)
            nc.vector.memset(bias_m1[:], -1.0)
            eps_sb = const.tile([P, 1], F32)
            nc.vector.memset(eps_sb[:], EPS)
            ones_f8 = const.tile([P, 2, 16], FP8)
            nc.vector.memset(ones_f8[:], 1.0)
            hn8 = persist.tile([P, 2, 2, N], FP8)
            vT_f8 = persist.tile([P, JT // 2, 2, 512], FP8)
            q_f8 = persist.tile([P, CT // 2, 2, NQ], FP8)
            scale_sb = persist.tile([P, CT], F32)
            bias_sb = persist.tile([P, CT], F32)

            for _rep in range(reps):
              if True:
                  # ---------------- phase 0: groupnorm stats ----------------
                  # red per ct = (mean_c, E_c[x^2]) [P, 2]; g4 is host-scaled by
                  # 1/GSIZE so the group matmul directly yields (mu_g, E_g[x^2]).
                  # ct0 computed on ACT (sum + sumsq accum), ct1-3 on DVE (bn_stats).
                  mrall = small.tile([4, 8], F32, tag="mrall")
                  for ct in (1, 2, 0, 3):
                      red = small.tile([P, 2], F32, tag="red", name=f"red_{ct}")
                      if ct == 0:
                          # ACT route: sums/sumsq accumulate while the x DMA streams
                          reds = small.tile([P, 4], F32, tag="reds")
                          redq = small.tile([P, 4], F32, tag="redq")
                          for jc in range(4):
                              cp = hnp.tile([P, 1024], F32, tag="cp", name=f"cp_{jc}")
                              nc.scalar.activation(
                                  cp[:], x_sb[:, ct, jc * 1024:(jc + 1) * 1024], AF.Copy,
                                  accum_out=reds[:, jc:jc + 1],
                              )
                              sq = hnp.tile([P, 1024], F32, tag="sq", name=f"sq_{jc}")
                              nc.scalar.activation(
                                  sq[:], x_sb[:, ct, jc * 1024:(jc + 1) * 1024], AF.Square,
                                  accum_out=redq[:, jc:jc + 1],
                              )
                          rsum = small.tile([P, 2], F32, tag="rsum")
                          nc.vector.reduce_sum(rsum[:, 0:1], reds[:], axis=mybir.AxisListType.X)
                          nc.vector.reduce_sum(rsum[:, 1:2], redq[:], axis=mybir.AxisListType.X)
                          nc.vector.tensor_scalar_mul(red[:], rsum[:], 1.0 / N)
                      elif ct < 3:
                          bnst = small.tile([P, JC, 6], F32, tag="bnst", name=f"bnst_{ct}")
                          for jc in range(JC):
                              nc.vector.bn_stats(bnst[:, jc, :], x_sb[:, ct, jc * 512:(jc + 1) * 512])
                          mv = small.tile([P, 2], F32, tag="mv", name=f"mv_{ct}")
                          nc.vector.bn_aggr(mv[:], bnst[:])
                          msq = small.tile([P, 1], F32, tag="msq", name=f"msq_{ct}")
                          nc.vector.tensor_tensor(msq[:], mv[:, 0:1], mv[:, 0:1], ALU.mult)
                          nc.vector.tensor_copy(red[:, 0:1], mv[:, 0:1])
                          nc.vector.tensor_tensor(red[:, 1:2], mv[:, 1:2], msq[:], ALU.add)
                      else:
                          # ct3 split: DVE bn_stats on the first 3/4, ACT
                          # sum/sumsq accumulators on the last 1/4
                          bnst = small.tile([P, 6, 6], F32, tag="bnst", name=f"bnst_{ct}")
                          for jc in range(6):
                              nc.vector.bn_stats(bnst[:, jc, :], x_sb[:, ct, jc * 512:(jc + 1) * 512])
                          s2q2 = small.tile([P, 2], F32, tag="s2q2")
                          cp3 = hnp.tile([P, 1024], F32, tag="cp", name="cp3")
                          nc.scalar.activation(
                              cp3[:], x_sb[:, ct, 3072:4096], AF.Copy,
                              accum_out=s2q2[:, 0:1],
                          )
                          sq3 = hnp.tile([P, 1024], F32, tag="sq", name="sq3")
                          nc.scalar.activation(
                              sq3[:], x_sb[:, ct, 3072:4096], AF.Square,
                              accum_out=s2q2[:, 1:2],
                          )
                          mv = small.tile([P, 2], F32, tag="mv", name=f"mv_{ct}")
                          nc.vector.bn_aggr(mv[:], bnst[:])
                          msq = small.tile([P, 1], F32, tag="msq", name=f"msq_{ct}")
                          nc.vector.tensor_tensor(msq[:], mv[:, 0:1], mv[:, 0:1], ALU.mult)
                          # E over 3072: (mean1, var1+mean1^2); combine with the
                          # 1024-position sums: red = 0.75*E1 + sums/4096
                          e1 = small.tile([P, 2], F32, tag="e1")
                          nc.vector.tensor_copy(e1[:, 0:1], mv[:, 0:1])
                          nc.vector.tensor_tensor(e1[:, 1:2], mv[:, 1:2], msq[:], ALU.add)
                          nc.vector.tensor_scalar(e1[:], e1[:], 0.75, None, ALU.mult)
                          nc.vector.tensor_scalar(s2q2[:], s2q2[:], 1.0 / N, None, ALU.mult)
                          nc.vector.tensor_tensor(red[:], e1[:], s2q2[:], ALU.add)
                      gps = ps.tile([4, 2], F32, tag="den", bufs=1, name=f"gps_{ct}")
                      nc.tensor.matmul(
                          gps[:], lhsT=g4_sb[:], rhs=red[:],
                          start=True, stop=True,
                      )
                      nc.vector.tensor_copy(mrall[:, ct:ct + 1], gps[:, 0:1])
                      nc.vector.tensor_copy(mrall[:, 4 + ct:5 + ct], gps[:, 1:2])
                  # mu = mrall[:, :4]; var = mrall[:, 4:] - mu^2 (batched)
                  musq = small.tile([4, 4], F32, tag="musq")
                  nc.vector.tensor_tensor(musq[:], mrall[:, 0:4], mrall[:, 0:4], ALU.mult)
                  var4 = small.tile([4, 4], F32, tag="var4")
                  nc.vector.tensor_tensor(var4[:], mrall[:, 4:8], musq[:], ALU.subtract)
                  # rstd = 1/sqrt(var + eps): ACT Sqrt (one extra table load in
                  # the prefix) + DVE reciprocal
                  std4 = small.tile([4, 4], F32, tag="var4", name="std4")
                  nc.scalar.activation(std4[:], var4[:], AF.Sqrt, bias=eps_sb[0:4, :])
                  nc.vector.reciprocal(mrall[:, 4:8], std4[:])
                  # one bcast matmul: [128, 8] = (mu | rstd) per channel
                  mrp = ps.tile([P, 8], F32, tag="den", bufs=1, name="mrp")
                  nc.tensor.matmul(
                      mrp[:], lhsT=g4t_sb[:], rhs=mrall[:],
                      start=True, stop=True,
                  )
                  # scale = gamma * rstd ; bias = beta - mu * scale (batched)
                  nc.vector.tensor_tensor(scale_sb[:], gam[:], mrp[:, 4:8], ALU.mult)
                  tb = small.tile([P, 4], F32, tag="tb")
                  nc.vector.tensor_tensor(tb[:], mrp[:, 0:4], scale_sb[:], ALU.mult)
                  nc.vector.tensor_tensor(bias_sb[:], bet[:], tb[:], ALU.subtract)

                  # ---------------- attention helpers ----------------
                  at2_all = [None] * ICH
                  rb_all = [None] * ICH
                  xqb_all = [None] * ICH
                  o2n_all = [None] * ICH

                  def emit_xqb(ich):
                      xqb = o2np.tile([P, CT, 512], F32, tag="xqb")
                      xqb_all[ich] = xqb
                      for ot in range(CT):
                          xq = finp.tile([P, 512], F32, tag="xq", name=f"xq_{ich}_{ot}")
                          nc.sync.dma_start(
                              xq[:], x_d[ot * P:(ot + 1) * P, ich * 512:(ich + 1) * 512]
                          )
                          nc.gpsimd.tensor_scalar(
                              xqb[:, ot, :], xq[:], bp4[:, ot:ot + 1], None, ALU.add
                          )

                  den_pend = []
                  den_emitted = {}

                  def emit_den_flush(den, last=False):
                      # den matmuls lag their exp by one scores-pair so the
                      # in-order PE stream never blocks on ACT
                      while den_pend and (last or len(den_pend) > 1):
                          at2 = den_pend.pop(0)
                          n = den_emitted.get(id(den), 0)
                          den_emitted[id(den)] = n + 1
                          nc.tensor.matmul(
                              den[:], lhsT=ones_f8[:, :, 0:1], rhs=at2[:],
                              perf_mode=DR,
                              start=(n == 0), stop=(last and not den_pend),
                          )

                  def emit_scores_t(ich, t, den, at2s):
                      pssc = ps.tile([P, 2, 512], F32, tag="sc", bufs=2, name="pssc")
                      for s in range(2):
                          jt = 2 * t + s
                          for kp in range(CT // 2):
                              nc.tensor.matmul(
                                  pssc[:, s, :], lhsT=hn8[:, kp, :, jt * P:(jt + 1) * P],
                                  rhs=q_f8[:, kp, :, ich * 512:(ich + 1) * 512],
                                  perf_mode=DR,
                                  start=(kp == 0), stop=(kp == CT // 2 - 1),
                              )
                      at2 = atp.tile([P, 2, 512], FP8, tag="at", name=f"at2_{ich}_{t}")
                      if len(at2s) == JT // 2:
                          at2s[t] = at2
                      else:
                          at2s.append(at2)
                      nc.scalar.activation(at2[:], pssc[:], AF.Exp, scale=SCALE, bias=bias_m1[:])
                      den_pend.append(at2)
                      emit_den_flush(den)

                  def emit_rb(ich, den):
                      rec = small.tile([1, 512], F32R, tag="rec")
                      with nc.allow_low_precision(reason="f32r softmax denom reciprocal"):
                          nc.vector.reciprocal(rec[:], den[:])
                      rbp = ps.tile([P, 512], F32, tag="den", bufs=1, name=f"rbp_{ich}")
                      nc.tensor.matmul(rbp[:], lhsT=ones_row[:], rhs=rec[:], start=True, stop=True)
                      rb = finp.tile([P, 512], F32, tag="rb")
                      rb_all[ich] = rb
                      nc.vector.tensor_scalar(rb[:], rbp[:], 1.0, None, ALU.mult)

                  o2pair_cur = [None]
                  attnv_pos = [0]

                  def emit_attnv_steps(ich, nsteps, tag="o2", bufs=1):
                      # emit the next `nsteps` attn@v chain matmuls for chunk
                      # `ich` (64 total: 4 ct-chains of 16), allocating a pair
                      # tile per ct-pair and converting each finished half
                      for _ in range(nsteps):
                          pos = attnv_pos[0]
                          if pos >= 4 * (JT // 2):
                              return
                          ct, t = pos // (JT // 2), pos % (JT // 2)
                          if pos == 0:
                              o2n_all[ich] = o2np.tile([P, 2, 2, 512], FP8, tag="o2n",
                                                       name=f"o2n_{ich}")
                          if ct % 2 == 0 and t == 0:
                              o2pair_cur[0] = ps.tile([P, 2, 512], F32, tag=tag, bufs=bufs,
                                                      name=f"o2_{ich}_{ct}")
                          o2t = o2pair_cur[0]
                          nc.tensor.matmul(
                              o2t[:, ct % 2, :], lhsT=vT_f8[:, t, :, ct * P:(ct + 1) * P],
                              rhs=at2_all[ich][t][:], perf_mode=DR,
                              start=(t == 0), stop=(t == JT // 2 - 1),
                          )
                          if t == JT // 2 - 1:
                              nc.vector.tensor_tensor(
                                  o2n_all[ich][:, ct // 2, ct % 2, :], o2t[:, ct % 2, :],
                                  rb_all[ich][:], ALU.mult
                              )
                          attnv_pos[0] = pos + 1

                  def emit_attnv_ct(ich, ct, tag="o2", bufs=1):
                      assert attnv_pos[0] == ct * (JT // 2)
                      emit_attnv_steps(ich, JT // 2, tag=tag, bufs=bufs)

                  def emit_proj_pair(ich, og, tag="o2", bufs=1, dma_eng=None):
                      p3 = ps.tile([P, 2, 512], F32, tag=tag, bufs=bufs, name="p3")
                      for s2 in range(2):
                          ot = 2 * og + s2
                          for kp in range(2):
                              nc.tensor.matmul(
                                  p3[:, s2, :], lhsT=wpt_f8[:, kp, :, ot * P:(ot + 1) * P],
                                  rhs=o2n_all[ich][:, kp], perf_mode=DR,
                                  start=(kp == 0), stop=(kp == 1),
                              )
                      fin = finp.tile([P, 2, 512], F32, tag="fin")
                      nc.vector.tensor_tensor(
                          fin[:], p3[:], xqb_all[ich][:, 2 * og:2 * og + 2, :], ALU.add
                      )
                      (dma_eng or nc.sync).dma_start(
                          out_d[og * 2 * P:(og + 1) * 2 * P,
                                ich * 512:(ich + 1) * 512].rearrange("(s p) n -> p s n", p=P),
                          fin[:],
                      )

                  # ------- fused phase: hn/k/q/v + ich0 scores per key chunk -------
                  # jc order interleaves early and late chunks so ich0 exps start
                  # as soon as hn/k of chunk 0 exist (q chunk 0 IS ich0's queries).
                  emit_xqb(0)
                  den0 = ps.tile([1, 512], F32, tag="den", bufs=1, name="den_0")
                  at2s0 = [None] * (JT // 2)
                  at2_all[0] = at2s0
                  jc_seq = (0, 4, 1, 5, 2, 6, 3, 7)

                  def emit_hn(jc):
                      jcs = slice(jc * 512, (jc + 1) * 512)
                      for kc in range(CT):
                          nc.gpsimd.tensor_scalar(
                              hn8[:, kc // 2, kc % 2, jcs], x_sb[:, kc, jcs],
                              scale_sb[:, kc:kc + 1], bias_sb[:, kc:kc + 1],
                              ALU.mult, ALU.add,
                          )

                  emit_hn(jc_seq[0])
                  for jci, jc in enumerate(jc_seq):
                      jcs = slice(jc * 512, (jc + 1) * 512)
                      # next chunk's hn goes ahead of this chunk's DVE convert so
                      # the in-order DVE stream stays one chunk ahead of the PE
                      if jci + 1 < len(jc_seq):
                          emit_hn(jc_seq[jci + 1])
                      if jc < ICH:
                          for cp2 in range(2):
                              pq = ps.tile([P, 2, 512], F32, tag=("sc" if cp2 == 0 else "o2"),
                                           bufs=(2 if cp2 == 0 else 1), name="pq")
                              for s2 in range(2):
                                  co = 2 * cp2 + s2
                                  for kp in range(2):
                                      nc.tensor.matmul(
                                          pq[:, s2, :], lhsT=wqt_f8[:, kp, :, co * P:(co + 1) * P],
                                          rhs=hn8[:, kp, :, jcs], perf_mode=DR,
                                          start=(kp == 0), stop=(kp == 1),
                                      )
                              for s2 in range(2):
                                  co = 2 * cp2 + s2
                                  if cp2 == 0:
                                      nc.vector.tensor_scalar(
                                          q_f8[:, cp2, s2, jcs], pq[:, s2, :],
                                          bq4[:, co:co + 1], None, ALU.add,
                                      )
                                  else:
                                      nc.scalar.activation(
                                          q_f8[:, cp2, s2, jcs], pq[:, s2, :],
                                          AF.Identity, bias=bq4[:, co:co + 1],
                                      )
                      # v: PSUM pair per jl-pair, merged Pool convert (bv folded
                      # into bp on host)
                      for vp2 in range(2):
                          pv = ps.tile([P, 2, 512], F32, tag=("sc" if vp2 == 0 else "o2"),
                                       bufs=(2 if vp2 == 0 else 1), name="pv")
                          for s2 in range(2):
                              jl = 2 * vp2 + s2
                              for kp in range(2):
                                  nc.tensor.matmul(
                                      pv[:, s2, :],
                                      lhsT=hn8[:, kp, :, jc * 512 + jl * P:jc * 512 + (jl + 1) * P],
                                      rhs=wvt_f8[:, kp], perf_mode=DR,
                                      start=(kp == 0), stop=(kp == 1),
                                  )
                          jt = jc * 4 + 2 * vp2
                          nc.vector.tensor_scalar(
                              vT_f8[:, jt // 2, 0:2, :], pv[:], 1.0, None, ALU.mult,
                          )
                      # ich0 scores for this key chunk (hn tiles just produced)
                      for tt in range(2):
                          emit_scores_t(0, 2 * jc + tt, den0, at2s0)
                  emit_den_flush(den0, last=True)
                  emit_rb(0, den0)

                  # ------- stages ich=1..3: scores(ich) + attnv(ich-1) -------
                  for ich in range(1, ICH):
                      emit_xqb(ich)
                      den = ps.tile([1, 512], F32, tag="den", bufs=1, name=f"den_{ich}")
                      at2s = []
                      at2_all[ich] = at2s
                      attnv_pos[0] = 0
                      last = ich == ICH - 1
                      if last:
                          # ich3's ct0 chain rides the spare 8th bank tile-major,
                          # tracking the exps, so the flush has one less chain
                          o2n_all[ich] = o2np.tile([P, 2, 2, 512], FP8, tag="o2n",
                                                   name=f"o2n_{ich}")
                          pp0 = ps.tile([P, 512], F32, tag="pp", bufs=1, name="pp0")
                      for t in range(JT // 2):
                          emit_scores_t(ich, t, den, at2s)
                          if last:
                              nc.tensor.matmul(
                                  pp0[:], lhsT=vT_f8[:, t, :, 0:P], rhs=at2s[t][:],
                                  perf_mode=DR,
                                  start=(t == 0), stop=(t == JT // 2 - 1),
                              )
                          if t >= 3:
                              emit_attnv_steps(ich - 1, 8)
                      emit_attnv_steps(ich - 1, 4 * (JT // 2))
                      emit_den_flush(den, last=True)
                      emit_proj_pair(ich - 1, 0)
                      emit_proj_pair(ich - 1, 1)
                      emit_rb(ich, den)

                  # ------- flush: remaining attnv(3) chains on the idle score
                  # pair slots -------
                  lich = ICH - 1
                  nc.vector.tensor_tensor(
                      o2n_all[lich][:, 0, 0, :], pp0[:], rb_all[lich][:], ALU.mult
                  )
                  scp = ps.tile([P, 2, 512], F32, tag="sc", bufs=2, name="scp")
                  scp2 = ps.tile([P, 2, 512], F32, tag="sc", bufs=2, name="scp2")
                  for ct in (1, 2, 3):
                      half = (ct - 1) % 2
                      o2t = scp if ct < 3 else scp2
                      for t in range(JT // 2):
                          nc.tensor.matmul(
                              o2t[:, half, :], lhsT=vT_f8[:, t, :, ct * P:(ct + 1) * P],
                              rhs=at2_all[lich][t][:], perf_mode=DR,
                              start=(t == 0), stop=(t == JT // 2 - 1),
                          )
                      nc.vector.tensor_tensor(
                          o2n_all[lich][:, ct // 2, ct % 2, :], o2t[:, half, :],
                          rb_all[lich][:], ALU.mult
                      )
                  emit_proj_pair(lich, 0)
                  emit_proj_pair(lich, 1, tag="sc", bufs=2, dma_eng=nc.scalar)
    return nc


_NC = None


def _get_nc():
    global _NC
    if _NC is None:
        _NC = build_nc()
    return _NC


def _make_in_maps(x, gamma, beta, wq, bq, wk, bk, wv, bv, wp, bp):
    x = np.ascontiguousarray(np.asarray(x, dtype=np.float32)).reshape(4, C, N)
    bf = ml_dtypes.bfloat16
    def pack8(w):
        return np.ascontiguousarray(
            np.asarray(w, np.float32).T.reshape(2, 2, P, 512).transpose(2, 0, 1, 3)
            .reshape(P, 4 * 512).astype(mybir.dt.np(FP8))
        )

    # bv folds into the proj bias (attention rows sum to 1); bk cancels in
    # softmax entirely.  Wk folds into the score matrix: scores^T =
    # hn^T (Wk^T Wq) hn + hn^T (Wk^T bq), so the device only sees
    # M = Wk^T Wq and cq = Wk^T bq and uses hn itself as the scores lhsT.
    bp_eff = np.asarray(bp, np.float32) + np.asarray(wp, np.float32) @ np.asarray(bv, np.float32)
    wk_f = np.asarray(wk, np.float32)
    m_qk = wk_f.T @ np.asarray(wq, np.float32)
    cq = wk_f.T @ np.asarray(bq, np.float32)

    g4i = np.zeros((P, 4), np.float32)
    for p in range(P):
        g4i[p, p // GSIZE] = 1.0
    g4 = g4i / GSIZE          # group-mean matmul (pre-scaled)
    g4t = np.ascontiguousarray(g4i.T)  # broadcast indicator (0/1)
    common = {
        "wqtf8": pack8(m_qk),
        "wvtf8": pack8(wv), "wptf8": pack8(wp),
        "gamma": np.asarray(gamma, np.float32), "beta": np.asarray(beta, np.float32),
        "bq": cq, "bp": bp_eff,
        "g4": g4, "g4t": g4t,
        "onesr": np.full((1, P), 1.0, np.float32),
    }
    in_maps = []
    for core in range(8):
        bidx, half = core // 2, core % 2
        xb = x[bidx]
        if half == 0:
            xp = xb
        else:
            xp = np.concatenate([xb[:, NQ:], xb[:, :NQ]], axis=1)
        xp = np.ascontiguousarray(xp)
        in_maps.append({"x": xp, "xbf": xp.astype(bf), **common})
    return in_maps


def run(inputs, trace=False):
    nc = _get_nc()
    in_maps = _make_in_maps(**inputs)
    res = run_bass_kernel_spmd(nc, in_maps, list(range(8)), trace=trace)
    out = np.empty((4, C, N), np.float32)
    for core in range(8):
        bidx, half = core // 2, core % 2
        o = res.results[core]["out"]
        if half == 0:
            out[bidx, :, :NQ] = o
        else:
            out[bidx, :, NQ:] = o
    return out.reshape(4, C, 64, 64), res


def kernel(**inputs):
    out, _ = run(inputs, trace=False)
    return out


# revision 83
# speedup vs baseline: 1.3077x; 1.0107x over previous
"""Trainium2 Bass kernel for nn_AttnBlock (GroupNorm -> 1x1 q/k/v -> attention -> proj -> residual).

Input x: [4, 512, 64, 64] f32. Sharding: 8 cores = 4 batches x 2 query-halves.
Each core gets its batch's full x (columns permuted so its query half is first),
computes GroupNorm + full k/vT, q for its half, attention over all 4096 keys for
its 2048 queries, proj + residual, and returns [512, 2048].

Numerics: GroupNorm stats and softmax normalization in f32; all matmuls in
fp8e4m3 with DoubleRow packing (2x PE throughput), accumulating in f32 PSUM.
exp(s - 1) keeps attention weights inside the e4m3 normal range.

Bias algebra (exact):
  - bk is dropped entirely: softmax over keys is invariant to the
    per-query-constant term (q+bq)@bk.
  - bv is folded into the output-proj bias on the host: rows of the
    normalized attention sum to 1, so attn@(v+bv) = attn@v_raw + bv and
    out = wp@(attn@v_raw)/den + (bp + wp@bv).
  - 1/den is applied at the attn@v PSUM->fp8 conversion (o2n = o2t * rb),
    so the final step is a single add of the residual+bias tile.

Wk is folded into the score matrix on the host: scores^T =
hn^T (Wk^T Wq) hn + hn^T (Wk^T bq), so the device never materializes k —
the fp8 hn itself is the scores lhsT (one fewer quantization on the k side).

Schedule: GN stats (ACT accum route for ct0 + a ct3 quarter, DVE bn_stats
for the rest) -> fused phase (per key chunk, order 0,4,1,5,...: hn on Pool,
qm/v pair matmuls + DVE/ACT converts, then ich0 scores/exp for that chunk)
-> stages ich=1..3: scores/exp(ich) with attnv(ich-1) chain steps
interleaved on PE (8 per slot), den matmuls lagging exps by two slots,
proj(ich-1) and rb(ich) at stage end; ich3's ct0 chain rides the spare
8th PSUM bank tile-major -> flush: remaining three chains on the idle
score slots, projection DMAs split across queues.

Engine/space legality (walrus): GPSIMD (Pool) must never touch PSUM — it
only runs SBUF-side work (hn, xqb, DMAs); all PSUM->fp8 conversions are on
DVE/ACT. PSUM banks: score/qkv pairs 2x2, qm/v spill pair 2, den+rbp 1,
ich3-ct0 chain 1.
"""

import numpy as np
import ml_dtypes

import concourse.bass as bass
import concourse.mybir as mybir
import concourse.tile as tile
from concourse.vector_clock import ScopedClock
from concourse.bass_utils import run_bass_kernel_spmd

F32 = mybir.dt.float32
F32R = mybir.dt.float32r
BF16 = mybir.dt.bfloat16
FP8 = mybir.dt.float8e4
AF = mybir.ActivationFunctionType
ALU = mybir.AluOpType

P = 128
C = 512          # channels
N = 4096         # spatial positions (64*64)
NQ = 2048        # queries per core (half)
CT = C // P      # 4 channel tiles
JC = N // 512    # 8 key chunks of 512
JT = N // P      # 32 key tiles of 128
ICH = NQ // 512  # 4 query chunks of 512
NUM_GROUPS = 16
GSIZE = C // NUM_GROUPS            # 32 channels per group
EPS = 1e-6
SCALE = float(C) ** -0.5
DR = mybir.MatmulPerfMode.DoubleRow


class PatchedTileContext(tile.TileContext):
    """walrus in this container accepts only ONE sync-wait per instruction;
    split extra waits onto same-engine NoOps placed just before the
    instruction (same queue => waits still execute before it)."""

    def _lower_ordered_insts(self, ordered):
        for bb_name, insts in list(ordered.items()):
            new_list = []
            for inst in insts:
                si = inst.sync_info
                if si is not None and si.on_wait and len(si.on_wait) > 1:
                    waits = list(si.on_wait)
                    for w in waits[:-1]:
                        nop = mybir.InstNoOp(
                            name=self.nc.get_next_instruction_name(),
                            engine=inst.engine,
                            sync_info=mybir.SyncInfo(on_wait=[w], on_update=[]),
                            bass_nofuse=True,
                        )
                        new_list.append(nop)
                    si.on_wait = [waits[-1]]
                new_list.append(inst)
            ordered[bb_name] = new_list
        super()._lower_ordered_insts(ordered)

    def _drain_and_barrier(self, tick_clock, wait_clock):
        drain_inst = self.nc.sync.drain()
        wait_clock.add_sem_waits(
            drain_inst.ins, ScopedClock({None: tick_clock.global_clock})
        )
        si = drain_inst.ins.sync_info
        if si is not None and si.on_wait and len(si.on_wait) > 1:
            waits = list(si.on_wait)
            si.on_wait = [waits[0]]
            for w in waits[1:]:
                d2 = self.nc.sync.drain()
                d2.ins.sync_info = mybir.SyncInfo(on_wait=[w], on_update=[])
        self.nc.all_engine_barrier()
        assert self.sems is not None
        popped = self.nc._tile_sem_poison_stack.pop()
        assert popped is self._sem_poison
        self.nc.clear_and_free_semaphores(list(self.sems.allocated().values()))
        self.nc.all_engine_barrier()


def build_nc(reps=1):
    nc = bass.Bass(name=f"attnblk_r{reps}")

    x_d = nc.dram_tensor("x", [C, N], F32, kind="ExternalInput")
    xbf_d = nc.dram_tensor("xbf", [C, N], BF16, kind="ExternalInput")
    wqtf8_d = nc.dram_tensor("wqtf8", [P, 4 * 512], FP8, kind="ExternalInput")
    wvtf8_d = nc.dram_tensor("wvtf8", [P, 4 * 512], FP8, kind="ExternalInput")
    wptf8_d = nc.dram_tensor("wptf8", [P, 4 * 512], FP8, kind="ExternalInput")
    gamma_d = nc.dram_tensor("gamma", [C], F32, kind="ExternalInput")
    beta_d = nc.dram_tensor("beta", [C], F32, kind="ExternalInput")
    bq_d = nc.dram_tensor("bq", [C], F32, kind="ExternalInput")
    bp_d = nc.dram_tensor("bp", [C], F32, kind="ExternalInput")
    g4_d = nc.dram_tensor("g4", [P, 4], F32, kind="ExternalInput")
    g4t_d = nc.dram_tensor("g4t", [4, P], F32, kind="ExternalInput")
    onesr_d = nc.dram_tensor("onesr", [1, P], F32R, kind="ExternalInput")
    out_d = nc.dram_tensor("out", [C, NQ], F32, kind="ExternalOutput")

    with PatchedTileContext(nc) as tc:
        with (
            tc.tile_pool(name="const", bufs=1) as const,
            tc.tile_pool(name="persist", bufs=1) as persist,
            tc.tile_pool(name="small", bufs=4) as small,
            tc.tile_pool(name="hnp", bufs=3) as hnp,
            tc.tile_pool(name="atp", bufs=34) as atp,
            tc.tile_pool(name="o2np", bufs=2) as o2np,
            tc.tile_pool(name="finp", bufs=3) as finp,
            tc.tile_pool(name="ps", bufs=1, space="PSUM") as ps,
        ):
            # ---------------- persistent tiles ----------------
            x_sb = persist.tile([P, CT, N], BF16)

            # SP queue: x ct0 (ACT stats route) first in fine chunks, GN matmul
            # consts, x ct3.
            for xh in range(4):
                nc.sync.dma_start(
                    x_sb[:, 0, xh * 1024:(xh + 1) * 1024],
                    xbf_d[0:P, xh * 1024:(xh + 1) * 1024],
                )
            g4_sb = const.tile([P, 4], F32)
            nc.sync.dma_start(g4_sb[:], g4_d[:, :])
            g4t_sb = const.tile([4, P], F32)
            nc.sync.dma_start(g4t_sb[:], g4t_d[:, :])
            ones_row = const.tile([1, P], F32R)
            nc.sync.dma_start(ones_row[:], onesr_d[:, :])
            for xh in range(2):
                nc.sync.dma_start(
                    x_sb[:, 3, xh * 2048:(xh + 1) * 2048],
                    xbf_d[3 * P:4 * P, xh * 2048:(xh + 1) * 2048],
                )
            # Pool queue: x ct1/ct2 (DVE bn_stats starts with ct1; ct1 in fine
            # chunks so the first bn_stats launches early), then consts.
            for xh in range(4):
                nc.gpsimd.dma_start(
                    x_sb[:, 1, xh * 1024:(xh + 1) * 1024],
                    xbf_d[P:2 * P, xh * 1024:(xh + 1) * 1024],
                )
            for xh in range(2):
                nc.gpsimd.dma_start(
                    x_sb[:, 2, xh * 2048:(xh + 1) * 2048],
                    xbf_d[2 * P:3 * P, xh * 2048:(xh + 1) * 2048],
                )
            gam = const.tile([P, CT], F32)
            nc.gpsimd.dma_start(gam[:], gamma_d[:].rearrange("(t p) -> p t", p=P))
            bet = const.tile([P, CT], F32)
            nc.gpsimd.dma_start(bet[:], beta_d[:].rearrange("(t p) -> p t", p=P))
            bq4 = const.tile([P, CT], F32)
            nc.gpsimd.dma_start(bq4[:], bq_d[:].rearrange("(t p) -> p t", p=P))
            bp4 = const.tile([P, CT], F32)
            nc.gpsimd.dma_start(bp4[:], bp_d[:].rearrange("(t p) -> p t", p=P))
            wqt_f8 = const.tile([P, 2, 2, C], FP8)
            nc.gpsimd.dma_start(wqt_f8[:], wqtf8_d[:, :].rearrange("p (kp s co) -> p kp s co", kp=2, s=2))
            wvt_f8 = const.tile([P, 2, 2, C], FP8)
            nc.gpsimd.dma_start(wvt_f8[:], wvtf8_d[:, :].rearrange("p (kp s co) -> p kp s co", kp=2, s=2))
            wpt_f8 = const.tile([P, 2, 2, C], FP8)
            nc.gpsimd.dma_start(wpt_f8[:], wptf8_d[:, :].rearrange("p (kp s co) -> p kp s co", kp=2, s=2))

            bias_m1 = const.tile([P, 1], F32)
            nc.vector.memset(bias_m1[:], -1.0)
            eps_sb = const.tile([P, 1], F32)
            nc.vector.memset(eps_sb[:], EPS)
            ones_f8 = const.tile([P, 2, 16], FP8)
            nc.vector.memset(ones_f8[:], 1.0)
            hn8 = persist.tile([P, 2, 2, N], FP8)
            vT_f8 = persist.tile([P, JT // 2, 2, 512], FP8)
            q_f8 = persist.tile([P, CT // 2, 2, NQ], FP8)
            scale_sb = persist.tile([P, CT], F32)
            bias_sb = persist.tile([P, CT], F32)

            for _rep in range(reps):
              if True:
                  # ---------------- phase 0: groupnorm stats ----------------
                  # red per ct = (mean_c, E_c[x^2]) [P, 2]; g4 is host-scaled by
                  # 1/GSIZE so the group matmul directly yields (mu_g, E_g[x^2]).
                  # ct0 computed on ACT (sum + sumsq accum), ct1-3 on DVE (bn_stats).
                  mrall = small.tile([4, 8], F32, tag="mrall")
                  for ct in (1, 2, 0, 3):
                      red = small.tile([P, 2], F32, tag="red", name=f"red_{ct}")
                      if ct == 0:
                          # ACT route: sums/sumsq accumulate while the x DMA streams
                          reds = small.tile([P, 4], F32, tag="reds")
                          redq = small.tile([P, 4], F32, tag="redq")
                          for jc in range(4):
                              cp = hnp.tile([P, 1024], F32, tag="cp", name=f"cp_{jc}")
                              nc.scalar.activation(
                                  cp[:], x_sb[:, ct, jc * 1024:(jc + 1) * 1024], AF.Copy,
                                  accum_out=reds[:, jc:jc + 1],
                              )
                              sq = hnp.tile([P, 1024], F32, tag="sq", name=f"sq_{jc}")
                              nc.scalar.activation(
                                  sq[:], x_sb[:, ct, jc * 1024:(jc + 1) * 1024], AF.Square,
                                  accum_out=redq[:, jc:jc + 1],
                              )
                          rsum = small.tile([P, 2], F32, tag="rsum")
                          nc.vector.reduce_sum(rsum[:, 0:1], reds[:], axis=mybir.AxisListType.X)
                          nc.vector.reduce_sum(rsum[:, 1:2], redq[:], axis=mybir.AxisListType.X)
                          nc.vector.tensor_scalar_mul(red[:], rsum[:], 1.0 / N)
                      elif ct < 3:
                          bnst = small.tile([P, JC, 6], F32, tag="bnst", name=f"bnst_{ct}")
                          for jc in range(JC):
                              nc.vector.bn_stats(bnst[:, jc, :], x_sb[:, ct, jc * 512:(jc + 1) * 512])
                          mv = small.tile([P, 2], F32, tag="mv", name=f"mv_{ct}")
                          nc.vector.bn_aggr(mv[:], bnst[:])
                          msq = small.tile([P, 1], F32, tag="msq", name=f"msq_{ct}")
                          nc.vector.tensor_tensor(msq[:], mv[:, 0:1], mv[:, 0:1], ALU.mult)
                          nc.vector.tensor_copy(red[:, 0:1], mv[:, 0:1])
                          nc.vector.tensor_tensor(red[:, 1:2], mv[:, 1:2], msq[:], ALU.add)
                      else:
                          # ct3 split: DVE bn_stats on the first 3/4, ACT
                          # sum/sumsq accumulators on the last 1/4
                          bnst = small.tile([P, 6, 6], F32, tag="bnst", name=f"bnst_{ct}")
                          for jc in range(6):
                              nc.vector.bn_stats(bnst[:, jc, :], x_sb[:, ct, jc * 512:(jc + 1) * 512])
                          s2q2 = small.tile([P, 2], F32, tag="s2q2")
                          cp3 = hnp.tile([P, 1024], F32, tag="cp", name="cp3")
                          nc.scalar.activation(
                              cp3[:], x_sb[:, ct, 3072:4096], AF.Copy,
                              accum_out=s2q2[:, 0:1],
                          )
                          sq3 = hnp.tile([P, 1024], F32, tag="sq", name="sq3")
                          nc.scalar.activation(
                              sq3[:], x_sb[:, ct, 3072:4096], AF.Square,
                              accum_out=s2q2[:, 1:2],
                          )
                          mv = small.tile([P, 2], F32, tag="mv", name=f"mv_{ct}")
                          nc.vector.bn_aggr(mv[:], bnst[:])
                          msq = small.tile([P, 1], F32, tag="msq", name=f"msq_{ct}")
                          nc.vector.tensor_tensor(msq[:], mv[:, 0:1], mv[:, 0:1], ALU.mult)
                          # E over 3072: (mean1, var1+mean1^2); combine with the
                          # 1024-position sums: red = 0.75*E1 + sums/4096
                          e1 = small.tile([P, 2], F32, tag="e1")
                          nc.vector.tensor_copy(e1[:, 0:1], mv[:, 0:1])
                          nc.vector.tensor_tensor(e1[:, 1:2], mv[:, 1:2], msq[:], ALU.add)
                          nc.vector.tensor_scalar(e1[:], e1[:], 0.75, None, ALU.mult)
                          nc.vector.tensor_scalar(s2q2[:], s2q2[:], 1.0 / N, None, ALU.mult)
                          nc.vector.tensor_tensor(red[:], e1[:], s2q2[:], ALU.add)
                      gps = ps.tile([4, 2], F32, tag="den", bufs=1, name=f"gps_{ct}")
                      nc.tensor.matmul(
                          gps[:], lhsT=g4_sb[:], rhs=red[:],
                          start=True, stop=True,
                      )
                      nc.vector.tensor_copy(mrall[:, ct:ct + 1], gps[:, 0:1])
                      nc.vector.tensor_copy(mrall[:, 4 + ct:5 + ct], gps[:, 1:2])
                  # mu = mrall[:, :4]; var = mrall[:, 4:] - mu^2 (batched)
                  musq = small.tile([4, 4], F32, tag="musq")
                  nc.vector.tensor_tensor(musq[:], mrall[:, 0:4], mrall[:, 0:4], ALU.mult)
                  var4 = small.tile([4, 4], F32, tag="var4")
                  nc.vector.tensor_tensor(var4[:], mrall[:, 4:8], musq[:], ALU.subtract)
                  # rstd = 1/sqrt(var + eps): ACT Sqrt (one extra table load in
                  # the prefix) + DVE reciprocal
                  std4 = small.tile([4, 4], F32, tag="var4", name="std4")
                  nc.scalar.activation(std4[:], var4[:], AF.Sqrt, bias=eps_sb[0:4, :])
                  nc.vector.reciprocal(mrall[:, 4:8], std4[:])
                  # one bcast matmul: [128, 8] = (mu | rstd) per channel
                  mrp = ps.tile([P, 8], F32, tag="den", bufs=1, name="mrp")
                  nc.tensor.matmul(
                      mrp[:], lhsT=g4t_sb[:], rhs=mrall[:],
                      start=True, stop=True,
                  )
                  # scale = gamma * rstd ; bias = beta - mu * scale (batched)
                  nc.vector.tensor_tensor(scale_sb[:], gam[:], mrp[:, 4:8], ALU.mult)
                  tb = small.tile([P, 4], F32, tag="tb")
                  nc.vector.tensor_tensor(tb[:], mrp[:, 0:4], scale_sb[:], ALU.mult)
                  nc.vector.tensor_tensor(bias_sb[:], bet[:], tb[:], ALU.subtract)

                  # ---------------- attention helpers ----------------
                  at2_all = [None] * ICH
                  rb_all = [None] * ICH
                  xqb_all = [None] * ICH
                  o2n_all = [None] * ICH

                  def emit_xqb(ich):
                      xqb = o2np.tile([P, CT, 512], F32, tag="xqb")
                      xqb_all[ich] = xqb
                      for ot in range(CT):
                          xq = finp.tile([P, 512], F32, tag="xq", name=f"xq_{ich}_{ot}")
                          nc.sync.dma_start(
                              xq[:], x_d[ot * P:(ot + 1) * P, ich * 512:(ich + 1) * 512]
                          )
                          nc.gpsimd.tensor_scalar(
                              xqb[:, ot, :], xq[:], bp4[:, ot:ot + 1], None, ALU.add
                          )

                  den_pend = []
                  den_emitted = {}

                  def emit_den_flush(den, last=False):
                      # den matmuls lag their exp by one scores-pair so the
                      # in-order PE stream never blocks on ACT
                      while den_pend and (last or len(den_pend) > 2):
                          at2 = den_pend.pop(0)
                          n = den_emitted.get(id(den), 0)
                          den_emitted[id(den)] = n + 1
                          nc.tensor.matmul(
                              den[:], lhsT=ones_f8[:, :, 0:1], rhs=at2[:],
                              perf_mode=DR,
                              start=(n == 0), stop=(last and not den_pend),
                          )

                  def emit_scores_t(ich, t, den, at2s):
                      pssc = ps.tile([P, 2, 512], F32, tag="sc", bufs=2, name="pssc")
                      for s in range(2):
                          jt = 2 * t + s
                          for kp in range(CT // 2):
                              nc.tensor.matmul(
                                  pssc[:, s, :], lhsT=hn8[:, kp, :, jt * P:(jt + 1) * P],
                                  rhs=q_f8[:, kp, :, ich * 512:(ich + 1) * 512],
                                  perf_mode=DR,
                                  start=(kp == 0), stop=(kp == CT // 2 - 1),
                              )
                      at2 = atp.tile([P, 2, 512], FP8, tag="at", name=f"at2_{ich}_{t}")
                      if len(at2s) == JT // 2:
                          at2s[t] = at2
                      else:
                          at2s.append(at2)
                      nc.scalar.activation(at2[:], pssc[:], AF.Exp, scale=SCALE, bias=bias_m1[:])
                      den_pend.append(at2)
                      emit_den_flush(den)

                  def emit_rb(ich, den):
                      rec = small.tile([1, 512], F32R, tag="rec")
                      with nc.allow_low_precision(reason="f32r softmax denom reciprocal"):
                          nc.vector.reciprocal(rec[:], den[:])
                      rbp = ps.tile([P, 512], F32, tag="den", bufs=1, name=f"rbp_{ich}")
                      nc.tensor.matmul(rbp[:], lhsT=ones_row[:], rhs=rec[:], start=True, stop=True)
                      rb = finp.tile([P, 512], F32, tag="rb")
                      rb_all[ich] = rb
                      nc.vector.tensor_scalar(rb[:], rbp[:], 1.0, None, ALU.mult)

                  o2pair_cur = [None]
                  attnv_pos = [0]

                  def emit_attnv_steps(ich, nsteps, tag="o2", bufs=1):
                      # emit the next `nsteps` attn@v chain matmuls for chunk
                      # `ich` (64 total: 4 ct-chains of 16), allocating a pair
                      # tile per ct-pair and converting each finished half
                      for _ in range(nsteps):
                          pos = attnv_pos[0]
                          if pos >= 4 * (JT // 2):
                              return
                          ct, t = pos // (JT // 2), pos % (JT // 2)
                          if pos == 0:
                              o2n_all[ich] = o2np.tile([P, 2, 2, 512], FP8, tag="o2n",
                                                       name=f"o2n_{ich}")
                          if ct % 2 == 0 and t == 0:
                              o2pair_cur[0] = ps.tile([P, 2, 512], F32, tag=tag, bufs=bufs,
                                                      name=f"o2_{ich}_{ct}")
                          o2t = o2pair_cur[0]
                          nc.tensor.matmul(
                              o2t[:, ct % 2, :], lhsT=vT_f8[:, t, :, ct * P:(ct + 1) * P],
                              rhs=at2_all[ich][t][:], perf_mode=DR,
                              start=(t == 0), stop=(t == JT // 2 - 1),
                          )
                          if t == JT // 2 - 1:
                              nc.vector.tensor_tensor(
                                  o2n_all[ich][:, ct // 2, ct % 2, :], o2t[:, ct % 2, :],
                                  rb_all[ich][:], ALU.mult
                              )
                          attnv_pos[0] = pos + 1

                  def emit_attnv_ct(ich, ct, tag="o2", bufs=1):
                      assert attnv_pos[0] == ct * (JT // 2)
                      emit_attnv_steps(ich, JT // 2, tag=tag, bufs=bufs)

                  def emit_proj_pair(ich, og, tag="o2", bufs=1, dma_eng=None):
                      p3 = ps.tile([P, 2, 512], F32, tag=tag, bufs=bufs, name="p3")
                      for s2 in range(2):
                          ot = 2 * og + s2
                          for kp in range(2):
                              nc.tensor.matmul(
                                  p3[:, s2, :], lhsT=wpt_f8[:, kp, :, ot * P:(ot + 1) * P],
                                  rhs=o2n_all[ich][:, kp], perf_mode=DR,
                                  start=(kp == 0), stop=(kp == 1),
                              )
                      fin = finp.tile([P, 2, 512], F32, tag="fin")
                      nc.vector.tensor_tensor(
                          fin[:], p3[:], xqb_all[ich][:, 2 * og:2 * og + 2, :], ALU.add
                      )
                      (dma_eng or nc.sync).dma_start(
                          out_d[og * 2 * P:(og + 1) * 2 * P,
                                ich * 512:(ich + 1) * 512].rearrange("(s p) n -> p s n", p=P),
                          fin[:],
                      )

                  # ------- fused phase: hn/k/q/v + ich0 scores per key chunk -------
                  # jc order interleaves early and late chunks so ich0 exps start
                  # as soon as hn/k of chunk 0 exist (q chunk 0 IS ich0's queries).
                  emit_xqb(0)
                  den0 = ps.tile([1, 512], F32, tag="den", bufs=1, name="den_0")
                  at2s0 = [None] * (JT // 2)
                  at2_all[0] = at2s0
                  jc_seq = (0, 4, 1, 5, 2, 6, 3, 7)

                  def emit_hn(jc):
                      jcs = slice(jc * 512, (jc + 1) * 512)
                      for kc in range(CT):
                          nc.gpsimd.tensor_scalar(
                              hn8[:, kc // 2, kc % 2, jcs], x_sb[:, kc, jcs],
                              scale_sb[:, kc:kc + 1], bias_sb[:, kc:kc + 1],
                              ALU.mult, ALU.add,
                          )

                  emit_hn(jc_seq[0])
                  for jci, jc in enumerate(jc_seq):
                      jcs = slice(jc * 512, (jc + 1) * 512)
                      # next chunk's hn goes ahead of this chunk's DVE convert so
                      # the in-order DVE stream stays one chunk ahead of the PE
                      if jci + 1 < len(jc_seq):
                          emit_hn(jc_seq[jci + 1])
                      if jc < ICH:
                          for cp2 in range(2):
                              pq = ps.tile([P, 2, 512], F32, tag=("sc" if cp2 == 0 else "o2"),
                                           bufs=(2 if cp2 == 0 else 1), name="pq")
                              for s2 in range(2):
                                  co = 2 * cp2 + s2
                                  for kp in range(2):
                                      nc.tensor.matmul(
                                          pq[:, s2, :], lhsT=wqt_f8[:, kp, :, co * P:(co + 1) * P],
                                          rhs=hn8[:, kp, :, jcs], perf_mode=DR,
                                          start=(kp == 0), stop=(kp == 1),
                                      )
                              for s2 in range(2):
                                  co = 2 * cp2 + s2
                                  if cp2 == 0:
                                      nc.vector.tensor_scalar(
                                          q_f8[:, cp2, s2, jcs], pq[:, s2, :],
                                          bq4[:, co:co + 1], None, ALU.add,
                                      )
                                  else:
                                      nc.scalar.activation(
                                          q_f8[:, cp2, s2, jcs], pq[:, s2, :],
                                          AF.Identity, bias=bq4[:, co:co + 1],
                                      )
                      # v: PSUM pair per jl-pair, merged Pool convert (bv folded
                      # into bp on host)
                      for vp2 in range(2):
                          pv = ps.tile([P, 2, 512], F32, tag=("sc" if vp2 == 0 else "o2"),
                                       bufs=(2 if vp2 == 0 else 1), name="pv")
                          for s2 in range(2):
                              jl = 2 * vp2 + s2
                              for kp in range(2):
                                  nc.tensor.matmul(
                                      pv[:, s2, :],
                                      lhsT=hn8[:, kp, :, jc * 512 + jl * P:jc * 512 + (jl + 1) * P],
                                      rhs=wvt_f8[:, kp], perf_mode=DR,
                                      start=(kp == 0), stop=(kp == 1),
                                  )
                          jt = jc * 4 + 2 * vp2
                          nc.vector.tensor_scalar(
                              vT_f8[:, jt // 2, 0:2, :], pv[:], 1.0, None, ALU.mult,
                          )
                      # ich0 scores for this key chunk (hn tiles just produced)
                      for tt in range(2):
                          emit_scores_t(0, 2 * jc + tt, den0, at2s0)
                  emit_den_flush(den0, last=True)
                  emit_rb(0, den0)

                  # ------- stages ich=1..3: scores(ich) + attnv(ich-1) -------
                  for ich in range(1, ICH):
                      emit_xqb(ich)
                      den = ps.tile([1, 512], F32, tag="den", bufs=1, name=f"den_{ich}")
                      at2s = []
                      at2_all[ich] = at2s
                      attnv_pos[0] = 0
                      last = ich == ICH - 1
                      if last:
                          # ich3's ct0 chain rides the spare 8th bank tile-major,
                          # tracking the exps, so the flush has one less chain
                          o2n_all[ich] = o2np.tile([P, 2, 2, 512], FP8, tag="o2n",
                                                   name=f"o2n_{ich}")
                          pp0 = ps.tile([P, 512], F32, tag="pp", bufs=1, name="pp0")
                      for t in range(JT // 2):
                          emit_scores_t(ich, t, den, at2s)
                          if last:
                              nc.tensor.matmul(
                                  pp0[:], lhsT=vT_f8[:, t, :, 0:P], rhs=at2s[t][:],
                                  perf_mode=DR,
                                  start=(t == 0), stop=(t == JT // 2 - 1),
                              )
                          if t >= 3:
                              emit_attnv_steps(ich - 1, 8)
                      emit_attnv_steps(ich - 1, 4 * (JT // 2))
                      emit_den_flush(den, last=True)
                      emit_proj_pair(ich - 1, 0)
                      emit_proj_pair(ich - 1, 1)
                      emit_rb(ich, den)

                  # ------- flush: remaining attnv(3) chains on the idle score
                  # pair slots -------
                  lich = ICH - 1
                  nc.vector.tensor_tensor(
                      o2n_all[lich][:, 0, 0, :], pp0[:], rb_all[lich][:], ALU.mult
                  )
                  scp = ps.tile([P, 2, 512], F32, tag="sc", bufs=2, name="scp")
                  scp2 = ps.tile([P, 2, 512], F32, tag="sc", bufs=2, name="scp2")
                  for ct in (1, 2, 3):
                      half = (ct - 1) % 2
                      o2t = scp if ct < 3 else scp2
                      for t in range(JT // 2):
                          nc.tensor.matmul(
                              o2t[:, half, :], lhsT=vT_f8[:, t, :, ct * P:(ct + 1) * P],
                              rhs=at2_all[lich][t][:], perf_mode=DR,
                              start=(t == 0), stop=(t == JT // 2 - 1),
                          )
                      nc.vector.tensor_tensor(
                          o2n_all[lich][:, ct // 2, ct % 2, :], o2t[:, half, :],
                          rb_all[lich][:], ALU.mult
                      )
                  emit_proj_pair(lich, 0)
                  emit_proj_pair(lich, 1, tag="sc", bufs=2, dma_eng=nc.scalar)
    return nc


_NC = None


def _get_nc():
    global _NC
    if _NC is None:
        _NC = build_nc()
    return _NC


def _make_in_maps(x, gamma, beta, wq, bq, wk, bk, wv, bv, wp, bp):
    x = np.ascontiguousarray(np.asarray(x, dtype=np.float32)).reshape(4, C, N)
    bf = ml_dtypes.bfloat16
    def pack8(w):
        return np.ascontiguousarray(
            np.asarray(w, np.float32).T.reshape(2, 2, P, 512).transpose(2, 0, 1, 3)
            .reshape(P, 4 * 512).astype(mybir.dt.np(FP8))
        )

    # bv folds into the proj bias (attention rows sum to 1); bk cancels in
    # softmax entirely.  Wk folds into the score matrix: scores^T =
    # hn^T (Wk^T Wq) hn + hn^T (Wk^T bq), so the device only sees
    # M = Wk^T Wq and cq = Wk^T bq and uses hn itself as the scores lhsT.
    bp_eff = np.asarray(bp, np.float32) + np.asarray(wp, np.float32) @ np.asarray(bv, np.float32)
    wk_f = np.asarray(wk, np.float32)
    m_qk = wk_f.T @ np.asarray(wq, np.float32)
    cq = wk_f.T @ np.asarray(bq, np.float32)

    g4i = np.zeros((P, 4), np.float32)
    for p in range(P):
        g4i[p, p // GSIZE] = 1.0
    g4 = g4i / GSIZE          # group-mean matmul (pre-scaled)
    g4t = np.ascontiguousarray(g4i.T)  # broadcast indicator (0/1)
    common = {
        "wqtf8": pack8(m_qk),
        "wvtf8": pack8(wv), "wptf8": pack8(wp),
        "gamma": np.asarray(gamma, np.float32), "beta": np.asarray(beta, np.float32),
        "bq": cq, "bp": bp_eff,
        "g4": g4, "g4t": g4t,
        "onesr": np.full((1, P), 1.0, np.float32),
    }
    in_maps = []
    for core in range(8):
        bidx, half = core // 2, core % 2
        xb = x[bidx]
        if half == 0:
            xp = xb
        else:
            xp = np.concatenate([xb[:, NQ:], xb[:, :NQ]], axis=1)
        xp = np.ascontiguousarray(xp)
        in_maps.append({"x": xp, "xbf": xp.astype(bf), **common})
    return in_maps


def run(inputs, trace=False):
    nc = _get_nc()
    in_maps = _make_in_maps(**inputs)
    res = run_bass_kernel_spmd(nc, in_maps, list(range(8)), trace=trace)
    out = np.empty((4, C, N), np.float32)
    for core in range(8):
        bidx, half = core // 2, core % 2
        o = res.results[core]["out"]
        if half == 0:
            out[bidx, :, :NQ] = o
        else:
            out[bidx, :, NQ:] = o
    return out.reshape(4, C, 64, 64), res


def kernel(**inputs):
    out, _ = run(inputs, trace=False)
    return out


# revision 88
# speedup vs baseline: 1.3166x; 1.0067x over previous
"""Trainium2 Bass kernel for nn_AttnBlock (GroupNorm -> 1x1 q/k/v -> attention -> proj -> residual).

Input x: [4, 512, 64, 64] f32. Sharding: 8 cores = 4 batches x 2 query-halves.
Each core gets its batch's full x (columns permuted so its query half is first),
computes GroupNorm + full k/vT, q for its half, attention over all 4096 keys for
its 2048 queries, proj + residual, and returns [512, 2048].

Numerics: GroupNorm stats and softmax normalization in f32; all matmuls in
fp8e4m3 with DoubleRow packing (2x PE throughput), accumulating in f32 PSUM.
exp(s - 1) keeps attention weights inside the e4m3 normal range.

Bias algebra (exact):
  - bk is dropped entirely: softmax over keys is invariant to the
    per-query-constant term (q+bq)@bk.
  - bv is folded into the output-proj bias on the host: rows of the
    normalized attention sum to 1, so attn@(v+bv) = attn@v_raw + bv and
    out = wp@(attn@v_raw)/den + (bp + wp@bv).
  - 1/den is applied at the attn@v PSUM->fp8 conversion (o2n = o2t * rb),
    so the final step is a single add of the residual+bias tile.

Wk is folded into the score matrix on the host: scores^T =
hn^T (Wk^T Wq) hn + hn^T (Wk^T bq), so the device never materializes k —
the fp8 hn itself is the scores lhsT (one fewer quantization on the k side).

Schedule: GN stats (ACT accum route for ct0 + a ct3 quarter, DVE bn_stats
for the rest) -> fused phase (per key chunk, order 0,4,1,5,...: hn on Pool,
qm/v pair matmuls + DVE/ACT converts, then ich0 scores/exp for that chunk)
-> stages ich=1..3: scores/exp(ich) with attnv(ich-1) chain steps
interleaved on PE (8 per slot), den matmuls lagging exps by two slots,
proj(ich-1) and rb(ich) at stage end; ich3's ct0 chain rides the spare
8th PSUM bank tile-major -> flush: remaining three chains on the idle
score slots, projection DMAs split across queues.

Engine/space legality (walrus): GPSIMD (Pool) must never touch PSUM — it
only runs SBUF-side work (hn, xqb, DMAs); all PSUM->fp8 conversions are on
DVE/ACT. PSUM banks: score/qkv pairs 2x2, qm/v spill pair 2, den+rbp 1,
ich3-ct0 chain 1.
"""

import numpy as np
import ml_dtypes

import concourse.bass as bass
import concourse.mybir as mybir
import concourse.tile as tile
from concourse.vector_clock import ScopedClock
from concourse.bass_utils import run_bass_kernel_spmd

F32 = mybir.dt.float32
F32R = mybir.dt.float32r
BF16 = mybir.dt.bfloat16
FP8 = mybir.dt.float8e4
AF = mybir.ActivationFunctionType
ALU = mybir.AluOpType

P = 128
C = 512          # channels
N = 4096         # spatial positions (64*64)
NQ = 2048        # queries per core (half)
CT = C // P      # 4 channel tiles
JC = N // 512    # 8 key chunks of 512
JT = N // P      # 32 key tiles of 128
ICH = NQ // 512  # 4 query chunks of 512
NUM_GROUPS = 16
GSIZE = C // NUM_GROUPS            # 32 channels per group
EPS = 1e-6
SCALE = float(C) ** -0.5
DR = mybir.MatmulPerfMode.DoubleRow


class PatchedTileContext(tile.TileContext):
    """walrus in this container accepts only ONE sync-wait per instruction;
    split extra waits onto same-engine NoOps placed just before the
    instruction (same queue => waits still execute before it)."""

    def _lower_ordered_insts(self, ordered):
        for bb_name, insts in list(ordered.items()):
            new_list = []
            for inst in insts:
                si = inst.sync_info
                if si is not None and si.on_wait and len(si.on_wait) > 1:
                    waits = list(si.on_wait)
                    for w in waits[:-1]:
                        nop = mybir.InstNoOp(
                            name=self.nc.get_next_instruction_name(),
                            engine=inst.engine,
                            sync_info=mybir.SyncInfo(on_wait=[w], on_update=[]),
                            bass_nofuse=True,
                        )
                        new_list.append(nop)
                    si.on_wait = [waits[-1]]
                new_list.append(inst)
            ordered[bb_name] = new_list
        super()._lower_ordered_insts(ordered)

    def _drain_and_barrier(self, tick_clock, wait_clock):
        drain_inst = self.nc.sync.drain()
        wait_clock.add_sem_waits(
            drain_inst.ins, ScopedClock({None: tick_clock.global_clock})
        )
        si = drain_inst.ins.sync_info
        if si is not None and si.on_wait and len(si.on_wait) > 1:
            waits = list(si.on_wait)
            si.on_wait = [waits[0]]
            for w in waits[1:]:
                d2 = self.nc.sync.drain()
                d2.ins.sync_info = mybir.SyncInfo(on_wait=[w], on_update=[])
        self.nc.all_engine_barrier()
        assert self.sems is not None
        popped = self.nc._tile_sem_poison_stack.pop()
        assert popped is self._sem_poison
        self.nc.clear_and_free_semaphores(list(self.sems.allocated().values()))
        self.nc.all_engine_barrier()


def build_nc(reps=1):
    nc = bass.Bass(name=f"attnblk_r{reps}")

    x_d = nc.dram_tensor("x", [C, N], F32, kind="ExternalInput")
    xbf_d = nc.dram_tensor("xbf", [C, N], BF16, kind="ExternalInput")
    wqtf8_d = nc.dram_tensor("wqtf8", [P, 4 * 512], FP8, kind="ExternalInput")
    wvtf8_d = nc.dram_tensor("wvtf8", [P, 4 * 512], FP8, kind="ExternalInput")
    wptf8_d = nc.dram_tensor("wptf8", [P, 4 * 512], FP8, kind="ExternalInput")
    gamma_d = nc.dram_tensor("gamma", [C], F32, kind="ExternalInput")
    beta_d = nc.dram_tensor("beta", [C], F32, kind="ExternalInput")
    bq_d = nc.dram_tensor("bq", [C], F32, kind="ExternalInput")
    bp_d = nc.dram_tensor("bp", [C], F32, kind="ExternalInput")
    g4_d = nc.dram_tensor("g4", [P, 4], F32, kind="ExternalInput")
    g4t_d = nc.dram_tensor("g4t", [4, P], F32, kind="ExternalInput")
    onesr_d = nc.dram_tensor("onesr", [1, P], F32R, kind="ExternalInput")
    out_d = nc.dram_tensor("out", [C, NQ], F32, kind="ExternalOutput")

    with PatchedTileContext(nc) as tc:
        with (
            tc.tile_pool(name="const", bufs=1) as const,
            tc.tile_pool(name="persist", bufs=1) as persist,
            tc.tile_pool(name="small", bufs=4) as small,
            tc.tile_pool(name="hnp", bufs=3) as hnp,
            tc.tile_pool(name="atp", bufs=34) as atp,
            tc.tile_pool(name="o2np", bufs=2) as o2np,
            tc.tile_pool(name="finp", bufs=3) as finp,
            tc.tile_pool(name="ps", bufs=1, space="PSUM") as ps,
        ):
            # ---------------- persistent tiles ----------------
            x_sb = persist.tile([P, CT, N], BF16)

            # SP queue: x ct0 (ACT stats route) first in fine chunks, GN matmul
            # consts, x ct3.
            for xh in range(4):
                nc.sync.dma_start(
                    x_sb[:, 0, xh * 1024:(xh + 1) * 1024],
                    xbf_d[0:P, xh * 1024:(xh + 1) * 1024],
                )
            g4_sb = const.tile([P, 4], F32)
            nc.sync.dma_start(g4_sb[:], g4_d[:, :])
            g4t_sb = const.tile([4, P], F32)
            nc.sync.dma_start(g4t_sb[:], g4t_d[:, :])
            ones_row = const.tile([1, P], F32R)
            nc.sync.dma_start(ones_row[:], onesr_d[:, :])
            for xh in range(2):
                nc.sync.dma_start(
                    x_sb[:, 3, xh * 2048:(xh + 1) * 2048],
                    xbf_d[3 * P:4 * P, xh * 2048:(xh + 1) * 2048],
                )
            # Pool queue: x ct1/ct2 (DVE bn_stats starts with ct1; ct1 in fine
            # chunks so the first bn_stats launches early), then consts.
            for xh in range(4):
                nc.gpsimd.dma_start(
                    x_sb[:, 1, xh * 1024:(xh + 1) * 1024],
                    xbf_d[P:2 * P, xh * 1024:(xh + 1) * 1024],
                )
            for xh in range(2):
                nc.gpsimd.dma_start(
                    x_sb[:, 2, xh * 2048:(xh + 1) * 2048],
                    xbf_d[2 * P:3 * P, xh * 2048:(xh + 1) * 2048],
                )
            gam = const.tile([P, CT], F32)
            nc.gpsimd.dma_start(gam[:], gamma_d[:].rearrange("(t p) -> p t", p=P))
            bet = const.tile([P, CT], F32)
            nc.gpsimd.dma_start(bet[:], beta_d[:].rearrange("(t p) -> p t", p=P))
            bq4 = const.tile([P, CT], F32)
            nc.gpsimd.dma_start(bq4[:], bq_d[:].rearrange("(t p) -> p t", p=P))
            bp4 = const.tile([P, CT], F32)
            nc.gpsimd.dma_start(bp4[:], bp_d[:].rearrange("(t p) -> p t", p=P))
            wqt_f8 = const.tile([P, 2, 2, C], FP8)
            nc.gpsimd.dma_start(wqt_f8[:], wqtf8_d[:, :].rearrange("p (kp s co) -> p kp s co", kp=2, s=2))
            wvt_f8 = const.tile([P, 2, 2, C], FP8)
            nc.gpsimd.dma_start(wvt_f8[:], wvtf8_d[:, :].rearrange("p (kp s co) -> p kp s co", kp=2, s=2))
            wpt_f8 = const.tile([P, 2, 2, C], FP8)
            nc.gpsimd.dma_start(wpt_f8[:], wptf8_d[:, :].rearrange("p (kp s co) -> p kp s co", kp=2, s=2))

            bias_m1 = const.tile([P, 1], F32)
            nc.vector.memset(bias_m1[:], -1.0)
            eps_sb = const.tile([P, 1], F32)
            nc.vector.memset(eps_sb[:], EPS)
            ones_f8 = const.tile([P, 2, 16], FP8)
            nc.vector.memset(ones_f8[:], 1.0)
            hn8 = persist.tile([P, 2, 2, N], FP8)
            vT_f8 = persist.tile([P, JT // 2, 2, 512], FP8)
            q_f8 = persist.tile([P, CT // 2, 2, NQ], FP8)
            scale_sb = persist.tile([P, CT], F32)
            bias_sb = persist.tile([P, CT], F32)

            for _rep in range(reps):
              if True:
                  # ---------------- phase 0: groupnorm stats ----------------
                  # red per ct = (mean_c, E_c[x^2]) [P, 2]; g4 is host-scaled by
                  # 1/GSIZE so the group matmul directly yields (mu_g, E_g[x^2]).
                  # ct0 computed on ACT (sum + sumsq accum), ct1-3 on DVE (bn_stats).
                  mrall = small.tile([4, 8], F32, tag="mrall")
                  for ct in (1, 2, 0, 3):
                      red = small.tile([P, 2], F32, tag="red", name=f"red_{ct}")
                      if ct == 0:
                          # ACT route: sums/sumsq accumulate while the x DMA streams
                          reds = small.tile([P, 4], F32, tag="reds")
                          redq = small.tile([P, 4], F32, tag="redq")
                          for jc in range(4):
                              cp = hnp.tile([P, 1024], F32, tag="cp", name=f"cp_{jc}")
                              nc.scalar.activation(
                                  cp[:], x_sb[:, ct, jc * 1024:(jc + 1) * 1024], AF.Copy,
                                  accum_out=reds[:, jc:jc + 1],
                              )
                              sq = hnp.tile([P, 1024], F32, tag="sq", name=f"sq_{jc}")
                              nc.scalar.activation(
                                  sq[:], x_sb[:, ct, jc * 1024:(jc + 1) * 1024], AF.Square,
                                  accum_out=redq[:, jc:jc + 1],
                              )
                          rsum = small.tile([P, 2], F32, tag="rsum")
                          nc.vector.reduce_sum(rsum[:, 0:1], reds[:], axis=mybir.AxisListType.X)
                          nc.vector.reduce_sum(rsum[:, 1:2], redq[:], axis=mybir.AxisListType.X)
                          nc.vector.tensor_scalar_mul(red[:], rsum[:], 1.0 / N)
                      elif ct < 3:
                          bnst = small.tile([P, JC, 6], F32, tag="bnst", name=f"bnst_{ct}")
                          for jc in range(JC):
                              nc.vector.bn_stats(bnst[:, jc, :], x_sb[:, ct, jc * 512:(jc + 1) * 512])
                          mv = small.tile([P, 2], F32, tag="mv", name=f"mv_{ct}")
                          nc.vector.bn_aggr(mv[:], bnst[:])
                          msq = small.tile([P, 1], F32, tag="msq", name=f"msq_{ct}")
                          nc.vector.tensor_tensor(msq[:], mv[:, 0:1], mv[:, 0:1], ALU.mult)
                          nc.vector.tensor_copy(red[:, 0:1], mv[:, 0:1])
                          nc.vector.tensor_tensor(red[:, 1:2], mv[:, 1:2], msq[:], ALU.add)
                      else:
                          # ct3 split: DVE bn_stats on the first 3/4, ACT
                          # sum/sumsq accumulators on the last 1/4
                          bnst = small.tile([P, 6, 6], F32, tag="bnst", name=f"bnst_{ct}")
                          for jc in range(6):
                              nc.vector.bn_stats(bnst[:, jc, :], x_sb[:, ct, jc * 512:(jc + 1) * 512])
                          s2q2 = small.tile([P, 2], F32, tag="s2q2")
                          cp3 = hnp.tile([P, 1024], F32, tag="cp", name="cp3")
                          nc.scalar.activation(
                              cp3[:], x_sb[:, ct, 3072:4096], AF.Copy,
                              accum_out=s2q2[:, 0:1],
                          )
                          sq3 = hnp.tile([P, 1024], F32, tag="sq", name="sq3")
                          nc.scalar.activation(
                              sq3[:], x_sb[:, ct, 3072:4096], AF.Square,
                              accum_out=s2q2[:, 1:2],
                          )
                          mv = small.tile([P, 2], F32, tag="mv", name=f"mv_{ct}")
                          nc.vector.bn_aggr(mv[:], bnst[:])
                          msq = small.tile([P, 1], F32, tag="msq", name=f"msq_{ct}")
                          nc.vector.tensor_tensor(msq[:], mv[:, 0:1], mv[:, 0:1], ALU.mult)
                          # E over 3072: (mean1, var1+mean1^2); combine with the
                          # 1024-position sums: red = 0.75*E1 + sums/4096
                          e1 = small.tile([P, 2], F32, tag="e1")
                          nc.vector.tensor_copy(e1[:, 0:1], mv[:, 0:1])
                          nc.vector.tensor_tensor(e1[:, 1:2], mv[:, 1:2], msq[:], ALU.add)
                          nc.vector.tensor_scalar(e1[:], e1[:], 0.75, None, ALU.mult)
                          nc.vector.tensor_scalar(s2q2[:], s2q2[:], 1.0 / N, None, ALU.mult)
                          nc.vector.tensor_tensor(red[:], e1[:], s2q2[:], ALU.add)
                      gps = ps.tile([4, 2], F32, tag="den", bufs=1, name=f"gps_{ct}")
                      nc.tensor.matmul(
                          gps[:], lhsT=g4_sb[:], rhs=red[:],
                          start=True, stop=True,
                      )
                      nc.vector.tensor_copy(mrall[:, ct:ct + 1], gps[:, 0:1])
                      nc.vector.tensor_copy(mrall[:, 4 + ct:5 + ct], gps[:, 1:2])
                  # mu = mrall[:, :4]; var = mrall[:, 4:] - mu^2 (batched)
                  musq = small.tile([4, 4], F32, tag="musq")
                  nc.vector.tensor_tensor(musq[:], mrall[:, 0:4], mrall[:, 0:4], ALU.mult)
                  var4 = small.tile([4, 4], F32, tag="var4")
                  nc.vector.tensor_tensor(var4[:], mrall[:, 4:8], musq[:], ALU.subtract)
                  # rstd = 1/sqrt(var + eps): ACT Sqrt (one extra table load in
                  # the prefix) + DVE reciprocal
                  std4 = small.tile([4, 4], F32, tag="var4", name="std4")
                  nc.scalar.activation(std4[:], var4[:], AF.Sqrt, bias=eps_sb[0:4, :])
                  nc.vector.reciprocal(mrall[:, 4:8], std4[:])
                  # one bcast matmul: [128, 8] = (mu | rstd) per channel
                  mrp = ps.tile([P, 8], F32, tag="den", bufs=1, name="mrp")
                  nc.tensor.matmul(
                      mrp[:], lhsT=g4t_sb[:], rhs=mrall[:],
                      start=True, stop=True,
                  )
                  # scale = gamma * rstd ; bias = beta - mu * scale (batched)
                  nc.vector.tensor_tensor(scale_sb[:], gam[:], mrp[:, 4:8], ALU.mult)
                  tb = small.tile([P, 4], F32, tag="tb")
                  nc.vector.tensor_tensor(tb[:], mrp[:, 0:4], scale_sb[:], ALU.mult)
                  nc.vector.tensor_tensor(bias_sb[:], bet[:], tb[:], ALU.subtract)

                  # ---------------- attention helpers ----------------
                  at2_all = [None] * ICH
                  rb_all = [None] * ICH
                  xqb_all = [None] * ICH
                  o2n_all = [None] * ICH

                  def emit_xqb(ich):
                      xqb = o2np.tile([P, CT, 512], F32, tag="xqb")
                      xqb_all[ich] = xqb
                      for ot in range(CT):
                          xq = finp.tile([P, 512], F32, tag="xq", name=f"xq_{ich}_{ot}")
                          nc.sync.dma_start(
                              xq[:], x_d[ot * P:(ot + 1) * P, ich * 512:(ich + 1) * 512]
                          )
                          nc.gpsimd.tensor_scalar(
                              xqb[:, ot, :], xq[:], bp4[:, ot:ot + 1], None, ALU.add
                          )

                  den_pend = []
                  den_emitted = {}

                  def emit_den_flush(den, last=False):
                      # den matmuls lag their exp by one scores-pair so the
                      # in-order PE stream never blocks on ACT
                      while den_pend and (last or len(den_pend) > 2):
                          at2 = den_pend.pop(0)
                          n = den_emitted.get(id(den), 0)
                          den_emitted[id(den)] = n + 1
                          nc.tensor.matmul(
                              den[:], lhsT=ones_f8[:, :, 0:1], rhs=at2[:],
                              perf_mode=DR,
                              start=(n == 0), stop=(last and not den_pend),
                          )

                  def emit_scores_t(ich, t, den, at2s):
                      pssc = ps.tile([P, 2, 512], F32, tag="sc", bufs=2, name="pssc")
                      for s in range(2):
                          jt = 2 * t + s
                          for kp in range(CT // 2):
                              nc.tensor.matmul(
                                  pssc[:, s, :], lhsT=hn8[:, kp, :, jt * P:(jt + 1) * P],
                                  rhs=q_f8[:, kp, :, ich * 512:(ich + 1) * 512],
                                  perf_mode=DR,
                                  start=(kp == 0), stop=(kp == CT // 2 - 1),
                              )
                      at2 = atp.tile([P, 2, 512], FP8, tag="at", name=f"at2_{ich}_{t}")
                      if len(at2s) == JT // 2:
                          at2s[t] = at2
                      else:
                          at2s.append(at2)
                      nc.scalar.activation(at2[:], pssc[:], AF.Exp, scale=SCALE, bias=bias_m1[:])
                      den_pend.append(at2)
                      emit_den_flush(den)

                  def emit_rb(ich, den):
                      rec = small.tile([1, 512], F32R, tag="rec")
                      with nc.allow_low_precision(reason="f32r softmax denom reciprocal"):
                          nc.vector.reciprocal(rec[:], den[:])
                      rbp = ps.tile([P, 512], F32, tag="den", bufs=1, name=f"rbp_{ich}")
                      nc.tensor.matmul(rbp[:], lhsT=ones_row[:], rhs=rec[:], start=True, stop=True)
                      rb = finp.tile([P, 512], F32, tag="rb")
                      rb_all[ich] = rb
                      nc.vector.tensor_scalar(rb[:], rbp[:], 1.0, None, ALU.mult)

                  o2pair_cur = [None]
                  attnv_pos = [0]

                  def emit_attnv_steps(ich, nsteps, tag="o2", bufs=1):
                      # emit the next `nsteps` attn@v chain matmuls for chunk
                      # `ich` (64 total: 4 ct-chains of 16), allocating a pair
                      # tile per ct-pair and converting each finished half
                      for _ in range(nsteps):
                          pos = attnv_pos[0]
                          if pos >= 4 * (JT // 2):
                              return
                          ct, t = pos // (JT // 2), pos % (JT // 2)
                          if pos == 0:
                              o2n_all[ich] = o2np.tile([P, 2, 2, 512], FP8, tag="o2n",
                                                       name=f"o2n_{ich}")
                          if ct % 2 == 0 and t == 0:
                              o2pair_cur[0] = ps.tile([P, 2, 512], F32, tag=tag, bufs=bufs,
                                                      name=f"o2_{ich}_{ct}")
                          o2t = o2pair_cur[0]
                          nc.tensor.matmul(
                              o2t[:, ct % 2, :], lhsT=vT_f8[:, t, :, ct * P:(ct + 1) * P],
                              rhs=at2_all[ich][t][:], perf_mode=DR,
                              start=(t == 0), stop=(t == JT // 2 - 1),
                          )
                          if t == JT // 2 - 1:
                              nc.vector.tensor_tensor(
                                  o2n_all[ich][:, ct // 2, ct % 2, :], o2t[:, ct % 2, :],
                                  rb_all[ich][:], ALU.mult
                              )
                          attnv_pos[0] = pos + 1

                  def emit_attnv_ct(ich, ct, tag="o2", bufs=1):
                      assert attnv_pos[0] == ct * (JT // 2)
                      emit_attnv_steps(ich, JT // 2, tag=tag, bufs=bufs)

                  def emit_proj_pair(ich, og, tag="o2", bufs=1, dma_eng=None,
                                     split_dma=False):
                      p3 = ps.tile([P, 2, 512], F32, tag=tag, bufs=bufs, name="p3")
                      for s2 in range(2):
                          ot = 2 * og + s2
                          for kp in range(2):
                              nc.tensor.matmul(
                                  p3[:, s2, :], lhsT=wpt_f8[:, kp, :, ot * P:(ot + 1) * P],
                                  rhs=o2n_all[ich][:, kp], perf_mode=DR,
                                  start=(kp == 0), stop=(kp == 1),
                              )
                      fin = finp.tile([P, 2, 512], F32, tag="fin")
                      nc.vector.tensor_tensor(
                          fin[:], p3[:], xqb_all[ich][:, 2 * og:2 * og + 2, :], ALU.add
                      )
                      if split_dma:
                          for s2, eng in ((0, dma_eng or nc.sync), (1, nc.gpsimd)):
                              eng.dma_start(
                                  out_d[(og * 2 + s2) * P:(og * 2 + s2 + 1) * P,
                                        ich * 512:(ich + 1) * 512],
                                  fin[:, s2, :],
                              )
                      else:
                          (dma_eng or nc.sync).dma_start(
                              out_d[og * 2 * P:(og + 1) * 2 * P,
                                    ich * 512:(ich + 1) * 512].rearrange("(s p) n -> p s n", p=P),
                              fin[:],
                          )

                  # ------- fused phase: hn/k/q/v + ich0 scores per key chunk -------
                  # jc order interleaves early and late chunks so ich0 exps start
                  # as soon as hn/k of chunk 0 exist (q chunk 0 IS ich0's queries).
                  emit_xqb(0)
                  den0 = ps.tile([1, 512], F32, tag="den", bufs=1, name="den_0")
                  at2s0 = [None] * (JT // 2)
                  at2_all[0] = at2s0
                  jc_seq = (0, 4, 1, 5, 2, 6, 3, 7)

                  def emit_hn(jc):
                      jcs = slice(jc * 512, (jc + 1) * 512)
                      for kc in range(CT):
                          nc.gpsimd.tensor_scalar(
                              hn8[:, kc // 2, kc % 2, jcs], x_sb[:, kc, jcs],
                              scale_sb[:, kc:kc + 1], bias_sb[:, kc:kc + 1],
                              ALU.mult, ALU.add,
                          )

                  emit_hn(jc_seq[0])
                  for jci, jc in enumerate(jc_seq):
                      jcs = slice(jc * 512, (jc + 1) * 512)
                      # next chunk's hn goes ahead of this chunk's DVE convert so
                      # the in-order DVE stream stays one chunk ahead of the PE
                      if jci + 1 < len(jc_seq):
                          emit_hn(jc_seq[jci + 1])
                      if jc < ICH:
                          for cp2 in range(2):
                              pq = ps.tile([P, 2, 512], F32, tag=("sc" if cp2 == 0 else "o2"),
                                           bufs=(2 if cp2 == 0 else 1), name="pq")
                              for s2 in range(2):
                                  co = 2 * cp2 + s2
                                  for kp in range(2):
                                      nc.tensor.matmul(
                                          pq[:, s2, :], lhsT=wqt_f8[:, kp, :, co * P:(co + 1) * P],
                                          rhs=hn8[:, kp, :, jcs], perf_mode=DR,
                                          start=(kp == 0), stop=(kp == 1),
                                      )
                              for s2 in range(2):
                                  co = 2 * cp2 + s2
                                  if cp2 == 0:
                                      nc.vector.tensor_scalar(
                                          q_f8[:, cp2, s2, jcs], pq[:, s2, :],
                                          bq4[:, co:co + 1], None, ALU.add,
                                      )
                                  else:
                                      nc.scalar.activation(
                                          q_f8[:, cp2, s2, jcs], pq[:, s2, :],
                                          AF.Identity, bias=bq4[:, co:co + 1],
                                      )
                      # v: PSUM pair per jl-pair, merged Pool convert (bv folded
                      # into bp on host)
                      for vp2 in range(2):
                          pv = ps.tile([P, 2, 512], F32, tag=("sc" if vp2 == 0 else "o2"),
                                       bufs=(2 if vp2 == 0 else 1), name="pv")
                          for s2 in range(2):
                              jl = 2 * vp2 + s2
                              for kp in range(2):
                                  nc.tensor.matmul(
                                      pv[:, s2, :],
                                      lhsT=hn8[:, kp, :, jc * 512 + jl * P:jc * 512 + (jl + 1) * P],
                                      rhs=wvt_f8[:, kp], perf_mode=DR,
                                      start=(kp == 0), stop=(kp == 1),
                                  )
                          jt = jc * 4 + 2 * vp2
                          nc.vector.tensor_scalar(
                              vT_f8[:, jt // 2, 0:2, :], pv[:], 1.0, None, ALU.mult,
                          )
                      # ich0 scores for this key chunk (hn tiles just produced)
                      for tt in range(2):
                          emit_scores_t(0, 2 * jc + tt, den0, at2s0)
                  emit_den_flush(den0, last=True)
                  emit_rb(0, den0)

                  # ------- stages ich=1..3: scores(ich) + attnv(ich-1) -------
                  for ich in range(1, ICH):
                      emit_xqb(ich)
                      den = ps.tile([1, 512], F32, tag="den", bufs=1, name=f"den_{ich}")
                      at2s = []
                      at2_all[ich] = at2s
                      attnv_pos[0] = 0
                      last = ich == ICH - 1
                      if last:
                          # ich3's ct0 chain rides the spare 8th bank tile-major,
                          # tracking the exps, so the flush has one less chain
                          o2n_all[ich] = o2np.tile([P, 2, 2, 512], FP8, tag="o2n",
                                                   name=f"o2n_{ich}")
                          pp0 = ps.tile([P, 512], F32, tag="pp", bufs=1, name="pp0")
                      for t in range(JT // 2):
                          emit_scores_t(ich, t, den, at2s)
                          if last:
                              nc.tensor.matmul(
                                  pp0[:], lhsT=vT_f8[:, t, :, 0:P], rhs=at2s[t][:],
                                  perf_mode=DR,
                                  start=(t == 0), stop=(t == JT // 2 - 1),
                              )
                          if t >= 3:
                              emit_attnv_steps(ich - 1, 8)
                      emit_attnv_steps(ich - 1, 4 * (JT // 2))
                      emit_den_flush(den, last=True)
                      emit_rb(ich, den)
                      emit_proj_pair(ich - 1, 0)
                      emit_proj_pair(ich - 1, 1)

                  # ------- flush: remaining attnv(3) chains on the idle score
                  # pair slots -------
                  lich = ICH - 1
                  nc.vector.tensor_tensor(
                      o2n_all[lich][:, 0, 0, :], pp0[:], rb_all[lich][:], ALU.mult
                  )
                  scp = ps.tile([P, 2, 512], F32, tag="sc", bufs=2, name="scp")
                  scp2 = ps.tile([P, 2, 512], F32, tag="sc", bufs=2, name="scp2")
                  for ct in (1, 2, 3):
                      half = (ct - 1) % 2
                      o2t = scp if ct < 3 else scp2
                      for t in range(JT // 2):
                          nc.tensor.matmul(
                              o2t[:, half, :], lhsT=vT_f8[:, t, :, ct * P:(ct + 1) * P],
                              rhs=at2_all[lich][t][:], perf_mode=DR,
                              start=(t == 0), stop=(t == JT // 2 - 1),
                          )
                      nc.vector.tensor_tensor(
                          o2n_all[lich][:, ct // 2, ct % 2, :], o2t[:, half, :],
                          rb_all[lich][:], ALU.mult
                      )
                  emit_proj_pair(lich, 0, split_dma=True)
                  emit_proj_pair(lich, 1, tag="sc", bufs=2, dma_eng=nc.scalar,
                                 split_dma=True)
    return nc


_NC = None


def _get_nc():
    global _NC
    if _NC is None:
        _NC = build_nc()
    return _NC


def _make_in_maps(x, gamma, beta, wq, bq, wk, bk, wv, bv, wp, bp):
    x = np.ascontiguousarray(np.asarray(x, dtype=np.float32)).reshape(4, C, N)
    bf = ml_dtypes.bfloat16
    def pack8(w):
        return np.ascontiguousarray(
            np.asarray(w, np.float32).T.reshape(2, 2, P, 512).transpose(2, 0, 1, 3)
            .reshape(P, 4 * 512).astype(mybir.dt.np(FP8))
        )

    # bv folds into the proj bias (attention rows sum to 1); bk cancels in
    # softmax entirely.  Wk folds into the score matrix: scores^T =
    # hn^T (Wk^T Wq) hn + hn^T (Wk^T bq), so the device only sees
    # M = Wk^T Wq and cq = Wk^T bq and uses hn itself as the scores lhsT.
    bp_eff = np.asarray(bp, np.float32) + np.asarray(wp, np.float32) @ np.asarray(bv, np.float32)
    wk_f = np.asarray(wk, np.float32)
    m_qk = wk_f.T @ np.asarray(wq, np.float32)
    cq = wk_f.T @ np.asarray(bq, np.float32)

    g4i = np.zeros((P, 4), np.float32)
    for p in range(P):
        g4i[p, p // GSIZE] = 1.0
    g4 = g4i / GSIZE          # group-mean matmul (pre-scaled)
    g4t = np.ascontiguousarray(g4i.T)  # broadcast indicator (0/1)
    common = {
        "wqtf8": pack8(m_qk),
        "wvtf8": pack8(wv), "wptf8": pack8(wp),
        "gamma": np.asarray(gamma, np.float32), "beta": np.asarray(beta, np.float32),
        "bq": cq, "bp": bp_eff,
        "g4": g4, "g4t": g4t,
        "onesr": np.full((1, P), 1.0, np.float32),
    }
    in_maps = []
    for core in range(8):
        bidx, half = core // 2, core % 2
        xb = x[bidx]
        if half == 0:
            xp = xb
        else:
            xp = np.concatenate([xb[:, NQ:], xb[:, :NQ]], axis=1)
        xp = np.ascontiguousarray(xp)
        in_maps.append({"x": xp, "xbf": xp.astype(bf), **common})
    return in_maps


def run(inputs, trace=False):
    nc = _get_nc()
    in_maps = _make_in_maps(**inputs)
    res = run_bass_kernel_spmd(nc, in_maps, list(range(8)), trace=trace)
    out = np.empty((4, C, N), np.float32)
    for core in range(8):
        bidx, half = core // 2, core % 2
        o = res.results[core]["out"]
        if half == 0:
            out[bidx, :, :NQ] = o
        else:
            out[bidx, :, NQ:] = o
    return out.reshape(4, C, 64, 64), res


def kernel(**inputs):
    out, _ = run(inputs, trace=False)
    return out


# revision 95
# speedup vs baseline: 1.3326x; 1.0122x over previous
"""Trainium2 Bass kernel for nn_AttnBlock (GroupNorm -> 1x1 q/k/v -> attention -> proj -> residual).

Input x: [4, 512, 64, 64] f32. Sharding: 8 cores = 4 batches x 2 query-halves.
Each core gets its batch's full x (columns permuted so its query half is first),
computes GroupNorm + full k/vT, q for its half, attention over all 4096 keys for
its 2048 queries, proj + residual, and returns [512, 2048].

Numerics: GroupNorm stats and softmax normalization in f32; all matmuls in
fp8e4m3 with DoubleRow packing (2x PE throughput), accumulating in f32 PSUM.
exp(s - 1) keeps attention weights inside the e4m3 normal range.

Bias algebra (exact):
  - bk is dropped entirely: softmax over keys is invariant to the
    per-query-constant term (q+bq)@bk.
  - bv is folded into the output-proj bias on the host: rows of the
    normalized attention sum to 1, so attn@(v+bv) = attn@v_raw + bv and
    out = wp@(attn@v_raw)/den + (bp + wp@bv).
  - 1/den is applied at the attn@v PSUM->fp8 conversion (o2n = o2t * rb),
    so the final step is a single add of the residual+bias tile.

Wk is folded into the score matrix on the host: scores^T =
hn^T (Wk^T Wq) hn + hn^T (Wk^T bq), so the device never materializes k —
the fp8 hn itself is the scores lhsT (one fewer quantization on the k side).

Schedule: GN stats (ACT accum route for ct0 + a ct3 quarter, DVE bn_stats
for the rest) -> fused phase (per key chunk, order 0,4,1,5,...: hn on Pool,
qm/v pair matmuls + DVE/ACT converts, then ich0 scores/exp for that chunk)
-> stages ich=1..3: scores/exp(ich) with attnv(ich-1) chain steps
interleaved on PE (8 per slot), den matmuls lagging exps by two slots,
proj(ich-1) and rb(ich) at stage end; ich3's ct0 chain rides the spare
8th PSUM bank tile-major -> flush: remaining three chains on the idle
score slots, projection DMAs split across queues.

Engine/space legality (walrus): GPSIMD (Pool) must never touch PSUM — it
only runs SBUF-side work (hn, xqb, DMAs); all PSUM->fp8 conversions are on
DVE/ACT. PSUM banks: score/qkv pairs 2x2, qm/v spill pair 2, den+rbp 1,
ich3-ct0 chain 1.
"""

import numpy as np
import ml_dtypes

import concourse.bass as bass
import concourse.mybir as mybir
import concourse.tile as tile
from concourse.vector_clock import ScopedClock
from concourse.bass_utils import run_bass_kernel_spmd

F32 = mybir.dt.float32
F32R = mybir.dt.float32r
BF16 = mybir.dt.bfloat16
FP8 = mybir.dt.float8e4
AF = mybir.ActivationFunctionType
ALU = mybir.AluOpType

P = 128
C = 512          # channels
N = 4096         # spatial positions (64*64)
NQ = 2048        # queries per core (half)
CT = C // P      # 4 channel tiles
JC = N // 512    # 8 key chunks of 512
JT = N // P      # 32 key tiles of 128
ICH = NQ // 512  # 4 query chunks of 512
NUM_GROUPS = 16
GSIZE = C // NUM_GROUPS            # 32 channels per group
EPS = 1e-6
SCALE = float(C) ** -0.5
DR = mybir.MatmulPerfMode.DoubleRow


class PatchedTileContext(tile.TileContext):
    """walrus in this container accepts only ONE sync-wait per instruction;
    split extra waits onto same-engine NoOps placed just before the
    instruction (same queue => waits still execute before it)."""

    def _lower_ordered_insts(self, ordered):
        for bb_name, insts in list(ordered.items()):
            new_list = []
            for inst in insts:
                si = inst.sync_info
                if si is not None and si.on_wait and len(si.on_wait) > 1:
                    waits = list(si.on_wait)
                    for w in waits[:-1]:
                        nop = mybir.InstNoOp(
                            name=self.nc.get_next_instruction_name(),
                            engine=inst.engine,
                            sync_info=mybir.SyncInfo(on_wait=[w], on_update=[]),
                            bass_nofuse=True,
                        )
                        new_list.append(nop)
                    si.on_wait = [waits[-1]]
                new_list.append(inst)
            ordered[bb_name] = new_list
        super()._lower_ordered_insts(ordered)

    def _drain_and_barrier(self, tick_clock, wait_clock):
        drain_inst = self.nc.sync.drain()
        wait_clock.add_sem_waits(
            drain_inst.ins, ScopedClock({None: tick_clock.global_clock})
        )
        si = drain_inst.ins.sync_info
        if si is not None and si.on_wait and len(si.on_wait) > 1:
            waits = list(si.on_wait)
            si.on_wait = [waits[0]]
            for w in waits[1:]:
                d2 = self.nc.sync.drain()
                d2.ins.sync_info = mybir.SyncInfo(on_wait=[w], on_update=[])
        self.nc.all_engine_barrier()
        assert self.sems is not None
        popped = self.nc._tile_sem_poison_stack.pop()
        assert popped is self._sem_poison
        self.nc.clear_and_free_semaphores(list(self.sems.allocated().values()))
        self.nc.all_engine_barrier()


def build_nc(reps=1):
    nc = bass.Bass(name=f"attnblk_r{reps}")

    x_d = nc.dram_tensor("x", [C, N], F32, kind="ExternalInput")
    xbf_d = nc.dram_tensor("xbf", [C, N], BF16, kind="ExternalInput")
    wqtf8_d = nc.dram_tensor("wqtf8", [P, 4 * 512], FP8, kind="ExternalInput")
    wvtf8_d = nc.dram_tensor("wvtf8", [P, 4 * 512], FP8, kind="ExternalInput")
    wptf8_d = nc.dram_tensor("wptf8", [P, 4 * 512], FP8, kind="ExternalInput")
    gamma_d = nc.dram_tensor("gamma", [C], F32, kind="ExternalInput")
    beta_d = nc.dram_tensor("beta", [C], F32, kind="ExternalInput")
    bq_d = nc.dram_tensor("bq", [C], F32, kind="ExternalInput")
    bp_d = nc.dram_tensor("bp", [C], F32, kind="ExternalInput")
    g4_d = nc.dram_tensor("g4", [P, 4], F32, kind="ExternalInput")
    g4t_d = nc.dram_tensor("g4t", [4, P], F32, kind="ExternalInput")
    onesr_d = nc.dram_tensor("onesr", [1, P], F32R, kind="ExternalInput")
    out_d = nc.dram_tensor("out", [C, NQ], F32, kind="ExternalOutput")

    with PatchedTileContext(nc) as tc:
        with (
            tc.tile_pool(name="const", bufs=1) as const,
            tc.tile_pool(name="persist", bufs=1) as persist,
            tc.tile_pool(name="small", bufs=4) as small,
            tc.tile_pool(name="hnp", bufs=3) as hnp,
            tc.tile_pool(name="atp", bufs=34) as atp,
            tc.tile_pool(name="o2np", bufs=2) as o2np,
            tc.tile_pool(name="finp", bufs=3) as finp,
            tc.tile_pool(name="ps", bufs=1, space="PSUM") as ps,
        ):
            # ---------------- persistent tiles ----------------
            x_sb = persist.tile([P, CT, N], BF16)

            # SP queue: x ct0 (ACT stats route) first in fine chunks, GN matmul
            # consts, x ct3.
            for xh in range(4):
                nc.sync.dma_start(
                    x_sb[:, 0, xh * 1024:(xh + 1) * 1024],
                    xbf_d[0:P, xh * 1024:(xh + 1) * 1024],
                )
            g4_sb = const.tile([P, 4], F32)
            nc.sync.dma_start(g4_sb[:], g4_d[:, :])
            g4t_sb = const.tile([4, P], F32)
            nc.sync.dma_start(g4t_sb[:], g4t_d[:, :])
            ones_row = const.tile([1, P], F32R)
            nc.sync.dma_start(ones_row[:], onesr_d[:, :])
            for xh in range(2):
                nc.sync.dma_start(
                    x_sb[:, 3, xh * 2048:(xh + 1) * 2048],
                    xbf_d[3 * P:4 * P, xh * 2048:(xh + 1) * 2048],
                )
            # Pool queue: x ct1/ct2 (DVE bn_stats starts with ct1; ct1 in fine
            # chunks so the first bn_stats launches early), then consts.
            for xh in range(4):
                nc.gpsimd.dma_start(
                    x_sb[:, 1, xh * 1024:(xh + 1) * 1024],
                    xbf_d[P:2 * P, xh * 1024:(xh + 1) * 1024],
                )
            for xh in range(2):
                nc.gpsimd.dma_start(
                    x_sb[:, 2, xh * 2048:(xh + 1) * 2048],
                    xbf_d[2 * P:3 * P, xh * 2048:(xh + 1) * 2048],
                )
            gam = const.tile([P, CT], F32)
            nc.gpsimd.dma_start(gam[:], gamma_d[:].rearrange("(t p) -> p t", p=P))
            bet = const.tile([P, CT], F32)
            nc.gpsimd.dma_start(bet[:], beta_d[:].rearrange("(t p) -> p t", p=P))
            bq4 = const.tile([P, CT], F32)
            nc.gpsimd.dma_start(bq4[:], bq_d[:].rearrange("(t p) -> p t", p=P))
            bp4 = const.tile([P, CT], F32)
            nc.gpsimd.dma_start(bp4[:], bp_d[:].rearrange("(t p) -> p t", p=P))
            wqt_f8 = const.tile([P, 2, 2, C], FP8)
            nc.gpsimd.dma_start(wqt_f8[:], wqtf8_d[:, :].rearrange("p (kp s co) -> p kp s co", kp=2, s=2))
            wvt_f8 = const.tile([P, 2, 2, C], FP8)
            nc.gpsimd.dma_start(wvt_f8[:], wvtf8_d[:, :].rearrange("p (kp s co) -> p kp s co", kp=2, s=2))
            wpt_f8 = const.tile([P, 2, 2, C], FP8)
            nc.gpsimd.dma_start(wpt_f8[:], wptf8_d[:, :].rearrange("p (kp s co) -> p kp s co", kp=2, s=2))

            bias_m1 = const.tile([P, 1], F32)
            nc.vector.memset(bias_m1[:], -1.0)
            eps_sb = const.tile([P, 1], F32)
            nc.vector.memset(eps_sb[:], EPS)
            ones_f8 = const.tile([P, 2, 16], FP8)
            nc.vector.memset(ones_f8[:], 1.0)
            hn8 = persist.tile([P, 2, 2, N], FP8)
            vT_f8 = persist.tile([P, JT // 2, 2, 512], FP8)
            q_f8 = persist.tile([P, CT // 2, 2, NQ], FP8)
            scale_sb = persist.tile([P, CT], F32)
            bias_sb = persist.tile([P, CT], F32)

            for _rep in range(reps):
              if True:
                  # ---------------- phase 0: groupnorm stats ----------------
                  # red per ct = (mean_c, E_c[x^2]) [P, 2]; g4 is host-scaled by
                  # 1/GSIZE so the group matmul directly yields (mu_g, E_g[x^2]).
                  # ct0 computed on ACT (sum + sumsq accum), ct1-3 on DVE (bn_stats).
                  mrall = small.tile([4, 8], F32, tag="mrall")
                  for ct in (1, 2, 0, 3):
                      red = small.tile([P, 2], F32, tag="red", name=f"red_{ct}")
                      if ct == 0:
                          # ACT route: sums/sumsq accumulate while the x DMA streams
                          reds = small.tile([P, 4], F32, tag="reds")
                          redq = small.tile([P, 4], F32, tag="redq")
                          for jc in range(4):
                              cp = hnp.tile([P, 1024], F32, tag="cp", name=f"cp_{jc}")
                              nc.scalar.activation(
                                  cp[:], x_sb[:, ct, jc * 1024:(jc + 1) * 1024], AF.Copy,
                                  accum_out=reds[:, jc:jc + 1],
                              )
                              sq = hnp.tile([P, 1024], F32, tag="sq", name=f"sq_{jc}")
                              nc.scalar.activation(
                                  sq[:], x_sb[:, ct, jc * 1024:(jc + 1) * 1024], AF.Square,
                                  accum_out=redq[:, jc:jc + 1],
                              )
                          rsum = small.tile([P, 2], F32, tag="rsum")
                          nc.vector.reduce_sum(rsum[:, 0:1], reds[:], axis=mybir.AxisListType.X)
                          nc.vector.reduce_sum(rsum[:, 1:2], redq[:], axis=mybir.AxisListType.X)
                          nc.vector.tensor_scalar_mul(red[:], rsum[:], 1.0 / N)
                      elif ct < 3:
                          bnst = small.tile([P, JC, 6], F32, tag="bnst", name=f"bnst_{ct}")
                          for jc in range(JC):
                              nc.vector.bn_stats(bnst[:, jc, :], x_sb[:, ct, jc * 512:(jc + 1) * 512])
                          mv = small.tile([P, 2], F32, tag="mv", name=f"mv_{ct}")
                          nc.vector.bn_aggr(mv[:], bnst[:])
                          msq = small.tile([P, 1], F32, tag="msq", name=f"msq_{ct}")
                          nc.vector.tensor_tensor(msq[:], mv[:, 0:1], mv[:, 0:1], ALU.mult)
                          nc.vector.tensor_copy(red[:, 0:1], mv[:, 0:1])
                          nc.vector.tensor_tensor(red[:, 1:2], mv[:, 1:2], msq[:], ALU.add)
                      else:
                          # ct3 split: DVE bn_stats on the first 3/4, ACT
                          # sum/sumsq accumulators on the last 1/4
                          bnst = small.tile([P, 6, 6], F32, tag="bnst", name=f"bnst_{ct}")
                          for jc in range(6):
                              nc.vector.bn_stats(bnst[:, jc, :], x_sb[:, ct, jc * 512:(jc + 1) * 512])
                          s2q2 = small.tile([P, 2], F32, tag="s2q2")
                          cp3 = hnp.tile([P, 1024], F32, tag="cp", name="cp3")
                          nc.scalar.activation(
                              cp3[:], x_sb[:, ct, 3072:4096], AF.Copy,
                              accum_out=s2q2[:, 0:1],
                          )
                          sq3 = hnp.tile([P, 1024], F32, tag="sq", name="sq3")
                          nc.scalar.activation(
                              sq3[:], x_sb[:, ct, 3072:4096], AF.Square,
                              accum_out=s2q2[:, 1:2],
                          )
                          mv = small.tile([P, 2], F32, tag="mv", name=f"mv_{ct}")
                          nc.vector.bn_aggr(mv[:], bnst[:])
                          msq = small.tile([P, 1], F32, tag="msq", name=f"msq_{ct}")
                          nc.vector.tensor_tensor(msq[:], mv[:, 0:1], mv[:, 0:1], ALU.mult)
                          # E over 3072: (mean1, var1+mean1^2); combine with the
                          # 1024-position sums: red = 0.75*E1 + sums/4096
                          e1 = small.tile([P, 2], F32, tag="e1")
                          nc.vector.tensor_copy(e1[:, 0:1], mv[:, 0:1])
                          nc.vector.tensor_tensor(e1[:, 1:2], mv[:, 1:2], msq[:], ALU.add)
                          nc.vector.tensor_scalar(e1[:], e1[:], 0.75, None, ALU.mult)
                          nc.vector.tensor_scalar(s2q2[:], s2q2[:], 1.0 / N, None, ALU.mult)
                          nc.vector.tensor_tensor(red[:], e1[:], s2q2[:], ALU.add)
                      gps = ps.tile([4, 2], F32, tag="den", bufs=1, name=f"gps_{ct}")
                      nc.tensor.matmul(
                          gps[:], lhsT=g4_sb[:], rhs=red[:],
                          start=True, stop=True,
                      )
                      nc.vector.tensor_copy(mrall[:, ct:ct + 1], gps[:, 0:1])
                      nc.vector.tensor_copy(mrall[:, 4 + ct:5 + ct], gps[:, 1:2])
                  # mu = mrall[:, :4]; var = mrall[:, 4:] - mu^2 (batched)
                  musq = small.tile([4, 4], F32, tag="musq")
                  nc.vector.tensor_tensor(musq[:], mrall[:, 0:4], mrall[:, 0:4], ALU.mult)
                  var4 = small.tile([4, 4], F32, tag="var4")
                  nc.vector.tensor_tensor(var4[:], mrall[:, 4:8], musq[:], ALU.subtract)
                  # rstd = 1/sqrt(var + eps): ACT Sqrt (one extra table load in
                  # the prefix) + DVE reciprocal
                  std4 = small.tile([4, 4], F32, tag="var4", name="std4")
                  nc.scalar.activation(std4[:], var4[:], AF.Sqrt, bias=eps_sb[0:4, :])
                  nc.vector.reciprocal(mrall[:, 4:8], std4[:])
                  # one bcast matmul: [128, 8] = (mu | rstd) per channel
                  mrp = ps.tile([P, 8], F32, tag="den", bufs=1, name="mrp")
                  nc.tensor.matmul(
                      mrp[:], lhsT=g4t_sb[:], rhs=mrall[:],
                      start=True, stop=True,
                  )
                  # scale = gamma * rstd ; bias = beta - mu * scale (batched)
                  nc.vector.tensor_tensor(scale_sb[:], gam[:], mrp[:, 4:8], ALU.mult)
                  tb = small.tile([P, 4], F32, tag="tb")
                  nc.vector.tensor_tensor(tb[:], mrp[:, 0:4], scale_sb[:], ALU.mult)
                  nc.vector.tensor_tensor(bias_sb[:], bet[:], tb[:], ALU.subtract)

                  # ---------------- attention helpers ----------------
                  at2_all = [None] * ICH
                  rb_all = [None] * ICH
                  xqb_all = [None] * ICH
                  o2n_all = [None] * ICH

                  def emit_xqb(ich):
                      xqb = o2np.tile([P, CT, 512], F32, tag="xqb")
                      xqb_all[ich] = xqb
                      for ot in range(CT):
                          xq = finp.tile([P, 512], F32, tag="xq", name=f"xq_{ich}_{ot}")
                          nc.sync.dma_start(
                              xq[:], x_d[ot * P:(ot + 1) * P, ich * 512:(ich + 1) * 512]
                          )
                          nc.gpsimd.tensor_scalar(
                              xqb[:, ot, :], xq[:], bp4[:, ot:ot + 1], None, ALU.add
                          )

                  den_pend = []
                  den_emitted = {}

                  def emit_den_flush(den, last=False):
                      # den matmuls lag their exp by one scores-pair so the
                      # in-order PE stream never blocks on ACT
                      while den_pend and (last or len(den_pend) > 14):
                          at2 = den_pend.pop(0)
                          n = den_emitted.get(id(den), 0)
                          den_emitted[id(den)] = n + 1
                          nc.tensor.matmul(
                              den[:], lhsT=ones_f8[:, :, 0:1], rhs=at2[:],
                              perf_mode=DR,
                              start=(n == 0), stop=(last and not den_pend),
                          )

                  def emit_scores_t(ich, t, den, at2s):
                      pssc = ps.tile([P, 2, 512], F32, tag="sc", bufs=2, name="pssc")
                      for s in range(2):
                          jt = 2 * t + s
                          for kp in range(CT // 2):
                              nc.tensor.matmul(
                                  pssc[:, s, :], lhsT=hn8[:, kp, :, jt * P:(jt + 1) * P],
                                  rhs=q_f8[:, kp, :, ich * 512:(ich + 1) * 512],
                                  perf_mode=DR,
                                  start=(kp == 0), stop=(kp == CT // 2 - 1),
                              )
                      at2 = atp.tile([P, 2, 512], FP8, tag="at", name=f"at2_{ich}_{t}")
                      if len(at2s) == JT // 2:
                          at2s[t] = at2
                      else:
                          at2s.append(at2)
                      nc.scalar.activation(at2[:], pssc[:], AF.Exp, scale=SCALE, bias=bias_m1[:])
                      den_pend.append(at2)
                      emit_den_flush(den)

                  def emit_rb(ich, den):
                      rec = small.tile([1, 512], F32R, tag="rec")
                      with nc.allow_low_precision(reason="f32r softmax denom reciprocal"):
                          nc.vector.reciprocal(rec[:], den[:])
                      rbp = ps.tile([P, 512], F32, tag="den", bufs=1, name=f"rbp_{ich}")
                      nc.tensor.matmul(rbp[:], lhsT=ones_row[:], rhs=rec[:], start=True, stop=True)
                      rb = finp.tile([P, 512], F32, tag="rb")
                      rb_all[ich] = rb
                      nc.vector.tensor_scalar(rb[:], rbp[:], 1.0, None, ALU.mult)

                  o2pair_cur = [None]
                  attnv_pos = [0]

                  def emit_attnv_steps(ich, nsteps, tag="o2", bufs=1):
                      # emit the next `nsteps` attn@v chain matmuls for chunk
                      # `ich` (64 total: 4 ct-chains of 16), allocating a pair
                      # tile per ct-pair and converting each finished half
                      for _ in range(nsteps):
                          pos = attnv_pos[0]
                          if pos >= 4 * (JT // 2):
                              return
                          ct, t = pos // (JT // 2), pos % (JT // 2)
                          if pos == 0:
                              o2n_all[ich] = o2np.tile([P, 2, 2, 512], FP8, tag="o2n",
                                                       name=f"o2n_{ich}")
                          if ct % 2 == 0 and t == 0:
                              o2pair_cur[0] = ps.tile([P, 2, 512], F32, tag=tag, bufs=bufs,
                                                      name=f"o2_{ich}_{ct}")
                          o2t = o2pair_cur[0]
                          nc.tensor.matmul(
                              o2t[:, ct % 2, :], lhsT=vT_f8[:, t, :, ct * P:(ct + 1) * P],
                              rhs=at2_all[ich][t][:], perf_mode=DR,
                              start=(t == 0), stop=(t == JT // 2 - 1),
                          )
                          if t == JT // 2 - 1:
                              nc.vector.tensor_tensor(
                                  o2n_all[ich][:, ct // 2, ct % 2, :], o2t[:, ct % 2, :],
                                  rb_all[ich][:], ALU.mult
                              )
                          attnv_pos[0] = pos + 1

                  def emit_attnv_ct(ich, ct, tag="o2", bufs=1):
                      assert attnv_pos[0] == ct * (JT // 2)
                      emit_attnv_steps(ich, JT // 2, tag=tag, bufs=bufs)

                  def emit_proj_pair(ich, og, tag="o2", bufs=1, dma_eng=None,
                                     split_dma=False):
                      p3 = ps.tile([P, 2, 512], F32, tag=tag, bufs=bufs, name="p3")
                      for s2 in range(2):
                          ot = 2 * og + s2
                          for kp in range(2):
                              nc.tensor.matmul(
                                  p3[:, s2, :], lhsT=wpt_f8[:, kp, :, ot * P:(ot + 1) * P],
                                  rhs=o2n_all[ich][:, kp], perf_mode=DR,
                                  start=(kp == 0), stop=(kp == 1),
                              )
                      fin = finp.tile([P, 2, 512], F32, tag="fin")
                      nc.vector.tensor_tensor(
                          fin[:], p3[:], xqb_all[ich][:, 2 * og:2 * og + 2, :], ALU.add
                      )
                      if split_dma:
                          for s2, eng in ((0, dma_eng or nc.sync), (1, nc.gpsimd)):
                              eng.dma_start(
                                  out_d[(og * 2 + s2) * P:(og * 2 + s2 + 1) * P,
                                        ich * 512:(ich + 1) * 512],
                                  fin[:, s2, :],
                              )
                      else:
                          (dma_eng or nc.sync).dma_start(
                              out_d[og * 2 * P:(og + 1) * 2 * P,
                                    ich * 512:(ich + 1) * 512].rearrange("(s p) n -> p s n", p=P),
                              fin[:],
                          )

                  # ------- fused phase: hn/k/q/v + ich0 scores per key chunk -------
                  # jc order interleaves early and late chunks so ich0 exps start
                  # as soon as hn/k of chunk 0 exist (q chunk 0 IS ich0's queries).
                  emit_xqb(0)
                  den0 = ps.tile([1, 512], F32, tag="den", bufs=1, name="den_0")
                  at2s0 = [None] * (JT // 2)
                  at2_all[0] = at2s0
                  jc_seq = (0, 4, 1, 5, 2, 6, 3, 7)

                  def emit_hn(jc):
                      jcs = slice(jc * 512, (jc + 1) * 512)
                      for kc in range(CT):
                          nc.gpsimd.tensor_scalar(
                              hn8[:, kc // 2, kc % 2, jcs], x_sb[:, kc, jcs],
                              scale_sb[:, kc:kc + 1], bias_sb[:, kc:kc + 1],
                              ALU.mult, ALU.add,
                          )

                  emit_hn(jc_seq[0])
                  for jci, jc in enumerate(jc_seq):
                      jcs = slice(jc * 512, (jc + 1) * 512)
                      # next chunk's hn goes ahead of this chunk's DVE convert so
                      # the in-order DVE stream stays one chunk ahead of the PE
                      if jci + 1 < len(jc_seq):
                          emit_hn(jc_seq[jci + 1])
                      if jc < ICH:
                          for cp2 in range(2):
                              pq = ps.tile([P, 2, 512], F32, tag=("sc" if cp2 == 0 else "o2"),
                                           bufs=(2 if cp2 == 0 else 1), name="pq")
                              for s2 in range(2):
                                  co = 2 * cp2 + s2
                                  for kp in range(2):
                                      nc.tensor.matmul(
                                          pq[:, s2, :], lhsT=wqt_f8[:, kp, :, co * P:(co + 1) * P],
                                          rhs=hn8[:, kp, :, jcs], perf_mode=DR,
                                          start=(kp == 0), stop=(kp == 1),
                                      )
                              for s2 in range(2):
                                  co = 2 * cp2 + s2
                                  if cp2 == 0:
                                      nc.vector.tensor_scalar(
                                          q_f8[:, cp2, s2, jcs], pq[:, s2, :],
                                          bq4[:, co:co + 1], None, ALU.add,
                                      )
                                  else:
                                      nc.scalar.activation(
                                          q_f8[:, cp2, s2, jcs], pq[:, s2, :],
                                          AF.Identity, bias=bq4[:, co:co + 1],
                                      )
                      # v: PSUM pair per jl-pair, merged Pool convert (bv folded
                      # into bp on host)
                      for vp2 in range(2):
                          pv = ps.tile([P, 2, 512], F32, tag=("sc" if vp2 == 0 else "o2"),
                                       bufs=(2 if vp2 == 0 else 1), name="pv")
                          for s2 in range(2):
                              jl = 2 * vp2 + s2
                              for kp in range(2):
                                  nc.tensor.matmul(
                                      pv[:, s2, :],
                                      lhsT=hn8[:, kp, :, jc * 512 + jl * P:jc * 512 + (jl + 1) * P],
                                      rhs=wvt_f8[:, kp], perf_mode=DR,
                                      start=(kp == 0), stop=(kp == 1),
                                  )
                          jt = jc * 4 + 2 * vp2
                          nc.vector.tensor_scalar(
                              vT_f8[:, jt // 2, 0:2, :], pv[:], 1.0, None, ALU.mult,
                          )
                      # ich0 scores for this key chunk (hn tiles just produced)
                      for tt in range(2):
                          emit_scores_t(0, 2 * jc + tt, den0, at2s0)
                  emit_den_flush(den0, last=True)
                  emit_rb(0, den0)

                  # ------- stages ich=1..3: scores(ich) + attnv(ich-1) -------
                  for ich in range(1, ICH):
                      emit_xqb(ich)
                      den = ps.tile([1, 512], F32, tag="den", bufs=1, name=f"den_{ich}")
                      at2s = []
                      at2_all[ich] = at2s
                      attnv_pos[0] = 0
                      last = ich == ICH - 1
                      if last:
                          # ich3's ct0 chain rides the spare 8th bank tile-major,
                          # tracking the exps, so the flush has one less chain
                          o2n_all[ich] = o2np.tile([P, 2, 2, 512], FP8, tag="o2n",
                                                   name=f"o2n_{ich}")
                          pp0 = ps.tile([P, 512], F32, tag="pp", bufs=1, name="pp0")
                      for t in range(JT // 2):
                          emit_scores_t(ich, t, den, at2s)
                          if last:
                              nc.tensor.matmul(
                                  pp0[:], lhsT=vT_f8[:, t, :, 0:P], rhs=at2s[t][:],
                                  perf_mode=DR,
                                  start=(t == 0), stop=(t == JT // 2 - 1),
                              )
                          if t >= 3:
                              emit_attnv_steps(ich - 1, 8)
                      emit_attnv_steps(ich - 1, 4 * (JT // 2))
                      emit_den_flush(den, last=True)
                      emit_rb(ich, den)
                      emit_proj_pair(ich - 1, 0)
                      emit_proj_pair(ich - 1, 1)

                  # ------- flush: remaining attnv(3) chains on the idle score
                  # pair slots -------
                  lich = ICH - 1
                  nc.vector.tensor_tensor(
                      o2n_all[lich][:, 0, 0, :], pp0[:], rb_all[lich][:], ALU.mult
                  )
                  scp = ps.tile([P, 2, 512], F32, tag="sc", bufs=2, name="scp")
                  scp2 = ps.tile([P, 2, 512], F32, tag="sc", bufs=2, name="scp2")
                  for ct in (1, 2, 3):
                      half = (ct - 1) % 2
                      o2t = scp if ct < 3 else scp2
                      for t in range(JT // 2):
                          nc.tensor.matmul(
                              o2t[:, half, :], lhsT=vT_f8[:, t, :, ct * P:(ct + 1) * P],
                              rhs=at2_all[lich][t][:], perf_mode=DR,
                              start=(t == 0), stop=(t == JT // 2 - 1),
                          )
                      nc.vector.tensor_tensor(
                          o2n_all[lich][:, ct // 2, ct % 2, :], o2t[:, half, :],
                          rb_all[lich][:], ALU.mult
                      )
                  emit_proj_pair(lich, 0, split_dma=True)
                  emit_proj_pair(lich, 1, tag="sc", bufs=2, dma_eng=nc.scalar,
                                 split_dma=True)
    return nc


_NC = None


def _get_nc():
    global _NC
    if _NC is None:
        _NC = build_nc()
    return _NC


def _make_in_maps(x, gamma, beta, wq, bq, wk, bk, wv, bv, wp, bp):
    x = np.ascontiguousarray(np.asarray(x, dtype=np.float32)).reshape(4, C, N)
    bf = ml_dtypes.bfloat16
    def pack8(w):
        return np.ascontiguousarray(
            np.asarray(w, np.float32).T.reshape(2, 2, P, 512).transpose(2, 0, 1, 3)
            .reshape(P, 4 * 512).astype(mybir.dt.np(FP8))
        )

    # bv folds into the proj bias (attention rows sum to 1); bk cancels in
    # softmax entirely.  Wk folds into the score matrix: scores^T =
    # hn^T (Wk^T Wq) hn + hn^T (Wk^T bq), so the device only sees
    # M = Wk^T Wq and cq = Wk^T bq and uses hn itself as the scores lhsT.
    bp_eff = np.asarray(bp, np.float32) + np.asarray(wp, np.float32) @ np.asarray(bv, np.float32)
    wk_f = np.asarray(wk, np.float32)
    m_qk = wk_f.T @ np.asarray(wq, np.float32)
    cq = wk_f.T @ np.asarray(bq, np.float32)

    g4i = np.zeros((P, 4), np.float32)
    for p in range(P):
        g4i[p, p // GSIZE] = 1.0
    g4 = g4i / GSIZE          # group-mean matmul (pre-scaled)
    g4t = np.ascontiguousarray(g4i.T)  # broadcast indicator (0/1)
    common = {
        "wqtf8": pack8(m_qk),
        "wvtf8": pack8(wv), "wptf8": pack8(wp),
        "gamma": np.asarray(gamma, np.float32), "beta": np.asarray(beta, np.float32),
        "bq": cq, "bp": bp_eff,
        "g4": g4, "g4t": g4t,
        "onesr": np.full((1, P), 1.0, np.float32),
    }
    in_maps = []
    for core in range(8):
        bidx, half = core // 2, core % 2
        xb = x[bidx]
        if half == 0:
            xp = xb
        else:
            xp = np.concatenate([xb[:, NQ:], xb[:, :NQ]], axis=1)
        xp = np.ascontiguousarray(xp)
        in_maps.append({"x": xp, "xbf": xp.astype(bf), **common})
    return in_maps


def run(inputs, trace=False):
    nc = _get_nc()
    in_maps = _make_in_maps(**inputs)
    res = run_bass_kernel_spmd(nc, in_maps, list(range(8)), trace=trace)
    out = np.empty((4, C, N), np.float32)
    for core in range(8):
        bidx, half = core // 2, core % 2
        o = res.results[core]["out"]
        if half == 0:
            out[bidx, :, :NQ] = o
        else:
            out[bidx, :, NQ:] = o
    return out.reshape(4, C, 64, 64), res


def kernel(**inputs):
    out, _ = run(inputs, trace=False)
    return out
